# revision 27
# baseline (speedup 1.0000x reference)
"""Trainium2 Bass kernel for nn_AttentionBlock (B=4, S=2048, D=1024, H=16, Dh=64).

Sharding: 8 cores = 4 batches x 2 head-groups (8 heads each). Every core runs
the same Bass program on different input slices. The output projection is
row-sharded over head-groups, so the host sums the two partial outputs per
batch (the "all-reduce" of the sharding hint, done on host since we return
full outputs anyway).

Per-core pipeline (all matmuls bf16):
  A) QKV projection: lhsT = X^T chunks [128,128], rhs = Wqkv [128,1536 cols]
     -> psum [128(S-tile), 512] per q/k/v. The psum is copied to SBUF bf16
     (DVE), RoPE applied with bf16 2x-mode DVE ops (4 independent temps),
     then PE-transposed per head into qT/kT [Dh, S] packs. V goes to SBUF
     augmented with a ones column (V_aug [Sk,65]).
  B) Attention, two heads (one partition-half each) interleaved per k-tile:
     sT[Sk-tile 128, Sq 512] = kT_tile.T @ qT_group (alternating PE row
     groups so LDWEIGHTS hides). exp on ScalarE (PSUM->SBUF bf16). The
     causal mask is a [128,128] upper-tri multiply on DVE applied to the
     exp output of diagonal tiles only. AV: x_aug^T[65, Sq] += V_aug.T @ pT
     with row 64 accumulating the softmax denominator Z for free.
     Normalization: copy the two Z rows -> [2,512], reciprocal on DVE, one
     K=2 broadcast matmul -> [128,512] psum, single DVE multiply into xT.
  C) Output projection: out[Sq,512] += xT_pair.T @ WoutPair, DMA to HBM.

Issue order software-pipelines groups: QKV chunks of group g+1 and output
chunks of group g-1 are interleaved between attention head-pairs of group g
so the PE always has filler work while ScalarE runs exp.
"""

import sys

for _p in ("/opt/pypackages", "/opt/trn_rl_repo"):
    if _p not in sys.path:
        sys.path.insert(0, _p)

import numpy as np
import ml_dtypes

BF16 = ml_dtypes.bfloat16

B, S, D, H, Dh = 4, 2048, 1024, 16, 64
HL = H // 2          # heads per core
NCORES = 8
ST = S // 128        # 16 S-tiles of 128
NG = S // 512        # 4 q-groups of 512
MAX_WAVELENGTH = 10000.0

_CACHE = {}


def _build_bass():
    import concourse.bass as bass
    import concourse.mybir as mybir
    from concourse import bacc
    from concourse.tile import TileContext
    from contextlib import ExitStack

    f32 = mybir.dt.float32
    bf16 = mybir.dt.bfloat16
    AT = mybir.ActivationFunctionType
    OP = mybir.AluOpType

    nc = bacc.Bacc("TRN2", target_bir_lowering=False)

    xt_d = nc.dram_tensor("xt", [D, S], bf16, kind="ExternalInput")
    wqkv_d = nc.dram_tensor("wqkv", [D, 3 * HL * Dh], bf16, kind="ExternalInput")
    wout_d = nc.dram_tensor("woutp", [4, 128, D], bf16, kind="ExternalInput")
    cos_d = nc.dram_tensor("cost", [S, Dh // 2], bf16, kind="ExternalInput")
    sin_d = nc.dram_tensor("sint", [S, Dh // 2], bf16, kind="ExternalInput")
    identb_d = nc.dram_tensor("identb", [128, 128], bf16, kind="ExternalInput")
    keept_d = nc.dram_tensor("keept", [128, 128], bf16, kind="ExternalInput")
    identf_d = nc.dram_tensor("identf", [128, 128], f32, kind="ExternalInput")
    out_d = nc.dram_tensor("out", [S, D], f32, kind="ExternalOutput")

    with TileContext(nc) as tc, ExitStack() as ctx:
        consts = ctx.enter_context(tc.tile_pool(name="consts", bufs=1))
        persist = ctx.enter_context(tc.tile_pool(name="persist", bufs=1))

        # DMA order matters: what a_chunk(0) needs comes first so the PE can
        # start while the rest of the inputs stream in.
        identb_sb = consts.tile([128, 128], bf16, tag="identb")
        nc.sync.dma_start(identb_sb, identb_d[:, :])
        cos_sb = consts.tile([128, ST, 32], bf16, tag="cos")
        nc.sync.dma_start(cos_sb, cos_d.rearrange("(t p) f -> p t f", p=128))
        sin_sb = consts.tile([128, ST, 32], bf16, tag="sin")
        nc.sync.dma_start(sin_sb, sin_d.rearrange("(t p) f -> p t f", p=128))
        wq_sb = consts.tile([128, 8, 1536], bf16, tag="wqkv")
        wq_r = wqkv_d.rearrange("(c p) n -> p c n", p=128)
        xt_full = consts.tile([128, 8, S], bf16, tag="xtf")
        xt_r = xt_d.rearrange("(c p) s -> p c s", p=128)
        nc.sync.dma_start(wq_sb[:, :, 0:512], wq_r[:, :, 0:512])
        nc.sync.dma_start(
            xt_full[:, :, 0:512], xt_r[:, :, 0:512]
        )
        nc.sync.dma_start(wq_sb[:, :, 512:1024], wq_r[:, :, 512:1024])
        nc.sync.dma_start(wq_sb[:, :, 1024:1536], wq_r[:, :, 1024:1536])
        for sc in range(1, 4):
            nc.sync.dma_start(
                xt_full[:, :, sc * 512 : (sc + 1) * 512],
                xt_r[:, :, sc * 512 : (sc + 1) * 512],
            )
        keept_sb = consts.tile([128, 128], bf16, tag="keept")
        nc.sync.dma_start(keept_sb, keept_d[:, :])
        identf_sb = consts.tile([128, 128], f32, tag="identf")
        nc.sync.dma_start(identf_sb, identf_d[:, :])
        wout_sb = consts.tile([128, 4, 1024], bf16, tag="wout")
        nc.sync.dma_start(wout_sb, wout_d.rearrange("q p n -> p q n"))

        qT = persist.tile([128, 4, S], bf16, tag="qT")
        kT = persist.tile([128, 4, S], bf16, tag="kT")
        xT = persist.tile([128, 4, S], bf16, tag="xT")
        vaug = persist.tile([128, HL, ST, Dh + 1], bf16, tag="vaug")
        nc.scalar.activation(
            vaug[:, :, :, Dh : Dh + 1],
            identb_sb[:, 0:1, None].to_broadcast((128, HL, ST, 1)),
            AT.Identity, bias=1.0, scale=0.0,
        )

        rw_pool = ctx.enter_context(tc.tile_pool(name="ropew", bufs=3))
        pt_pool = ctx.enter_context(tc.tile_pool(name="ptp", bufs=4))
        nrm_pool = ctx.enter_context(tc.tile_pool(name="nrm", bufs=2))
        xus_pool = ctx.enter_context(tc.tile_pool(name="xus", bufs=5))
        out_pool = ctx.enter_context(tc.tile_pool(name="outp", bufs=3))
        psQ = ctx.enter_context(tc.tile_pool(name="psQ", bufs=2, space="PSUM"))
        psS = ctx.enter_context(tc.tile_pool(name="psS", bufs=2, space="PSUM"))
        psX = ctx.enter_context(tc.tile_pool(name="psX", bufs=2, space="PSUM"))

        def a_proj(si):
            # q/k projection + RoPE (DVE) + v projection.  The PE-transposes
            # of the rotated q/k are issued later (a_tail) so other PE work
            # can fill the RoPE latency.
            cos_b = cos_sb[:, si, None, :].to_broadcast((128, HL, 32))
            sin_b = sin_sb[:, si, None, :].to_broadcast((128, HL, 32))
            rots = []
            for qkv in (0, 1):
                ps = psQ.tile([128, 512], f32, tag="pqkv", name=f"ps{qkv}")
                for c in range(8):
                    nc.tensor.matmul(
                        ps, xt_full[:, c, si * 128 : (si + 1) * 128],
                        wq_sb[:, c, qkv * 512 : qkv * 512 + 512],
                        start=(c == 0), stop=(c == 7),
                    )
                xsb = rw_pool.tile([128, HL, Dh], bf16, tag="xsb")
                nc.vector.tensor_copy(xsb, ps.rearrange("p (h d) -> p h d", h=HL))
                x1, x2 = xsb[:, :, 0:32], xsb[:, :, 32:64]
                rot = rw_pool.tile([128, HL, Dh], bf16, tag="rot")
                t1 = rw_pool.tile([128, HL, 32], bf16, tag="t1")
                t2 = rw_pool.tile([128, HL, 32], bf16, tag="t2")
                t3 = rw_pool.tile([128, HL, 32], bf16, tag="t3")
                t4 = rw_pool.tile([128, HL, 32], bf16, tag="t4")
                nc.vector.tensor_tensor(t1, x1, cos_b, OP.mult)
                nc.vector.tensor_tensor(t2, x2, sin_b, OP.mult)
                nc.vector.tensor_tensor(t3, x1, sin_b, OP.mult)
                nc.vector.tensor_tensor(t4, x2, cos_b, OP.mult)
                nc.vector.tensor_tensor(rot[:, :, 0:32], t1, t2, OP.subtract)
                nc.vector.tensor_tensor(rot[:, :, 32:64], t3, t4, OP.add)
                rots.append(rot)
            ps_v = psQ.tile([128, 512], f32, tag="pqkv", name="ps_v")
            for c in range(8):
                nc.tensor.matmul(
                    ps_v, xt_full[:, c, si * 128 : (si + 1) * 128],
                    wq_sb[:, c, 1024:1536],
                    start=(c == 0), stop=(c == 7),
                )
            nc.vector.tensor_copy(
                vaug[:, :, si, 0:Dh],
                ps_v.rearrange("p (h d) -> p h d", h=HL),
            )
            return rots

        def a_tail(si, rots):
            for dstT, rot in zip((qT, kT), rots):
                rotf = rot.rearrange("p h d -> p (h d)")
                ps_t = psQ.tile([128, 512], f32, tag="pqkv", name="ps_t")
                for jj in range(4):
                    nc.tensor.matmul(
                        ps_t[:, jj * 128 : (jj + 1) * 128],
                        rotf[:, jj * 128 : (jj + 1) * 128],
                        identb_sb, start=True, stop=True,
                    )
                nc.vector.tensor_copy(
                    dstT[:, :, si * 128 : (si + 1) * 128],
                    ps_t.rearrange("p (j s) -> p j s", j=4),
                )

        def b_pair(g, hh, zsb8):
            nj = 4 * (g + 1)
            px = [
                psX.tile([Dh + 1, 512], f32, tag="psx", name=f"px{hp}")
                for hp in range(2)
            ]
            for jj in range(0, nj, 2):
                ss = []
                for j in (jj, jj + 1):
                    diag = j >= 4 * g
                    c0 = 128 * (j - 4 * g) if diag else 0
                    ps_s = psS.tile([128, 2, 512], f32, tag="pss")
                    for hp in range(2):
                        nc.tensor.matmul(
                            ps_s[:, hp, c0:512],
                            kT[64 * hp : 64 * hp + 64, hh,
                               j * 128 : (j + 1) * 128],
                            qT[64 * hp : 64 * hp + 64, hh,
                               g * 512 + c0 : (g + 1) * 512],
                            start=True, stop=True,
                        )
                    pt = pt_pool.tile([128, 2, 512], bf16, tag="pt")
                    nc.scalar.activation(pt[:, :, c0:512], ps_s[:, :, c0:512],
                                         AT.Exp)
                    if diag:
                        # split the triangular masks across DVE and the
                        # otherwise-idle GpSimd engine
                        nc.vector.tensor_tensor(
                            pt[:, 0, c0 : c0 + 128],
                            pt[:, 0, c0 : c0 + 128],
                            keept_sb, OP.mult,
                        )
                        nc.gpsimd.tensor_tensor(
                            pt[:, 1, c0 : c0 + 128],
                            pt[:, 1, c0 : c0 + 128],
                            keept_sb, OP.mult,
                        )
                    ss.append((j, c0, pt))
                for j, c0, pt in ss:
                    for hp in range(2):
                        nc.tensor.matmul(
                            px[hp][:, c0:512],
                            vaug[:, 2 * hh + hp, j, :], pt[:, hp, c0:512],
                            start=(j == 0), stop=(j == nj - 1),
                        )
            # stash Z rows into zsb8 (partition placement needs the gpsimd DMA)
            # and the unnormalized x into xus; normalization happens at the
            # group tail once all 8 heads' Z are in place.
            zta = nrm_pool.tile([1, 512], f32, tag="zta")
            nc.vector.tensor_copy(zta, px[0][Dh : Dh + 1, :])
            ztb = nrm_pool.tile([1, 512], f32, tag="ztb")
            nc.vector.tensor_copy(ztb, px[1][Dh : Dh + 1, :])
            nc.gpsimd.dma_start(zsb8[2 * hh : 2 * hh + 1, :], zta)
            nc.gpsimd.dma_start(zsb8[2 * hh + 1 : 2 * hh + 2, :], ztb)
            xus = xus_pool.tile([128, 512], bf16, tag="xus", name=f"xus{hh}")
            nc.vector.tensor_copy(xus[0:64, :], px[0][0:Dh, :])
            nc.vector.tensor_copy(xus[64:128, :], px[1][0:Dh, :])
            return xus

        def b_norm(g, zsb8, xus_l):
            # Z [8,512] -> columns [128,(m,h)] via K=8 transpose matmuls ->
            # reciprocal on fat partitions -> per-pair broadcast matmuls with
            # a stride-0 (free-dim broadcast) lhsT -> one DVE multiply per pair
            zc_t = psX.tile([128, 512], f32, tag="psx", name="zc_t")
            zcv = zc_t[:, 0:32].rearrange("p (m h) -> p m h", m=4)
            for m in range(4):
                nc.tensor.matmul(
                    zcv[:, m, :], zsb8[:, m * 128 : (m + 1) * 128],
                    identf_sb[0:HL, 0:HL], start=True, stop=True,
                )
            rcol8 = nrm_pool.tile([128, 4, HL], f32, tag="rcol")
            nc.vector.reciprocal(rcol8, zcv)
            for hh in range(4):
                bc_t = psX.tile([128, 512], f32, tag="psx", name="bc_t")
                for m in range(4):
                    for hp in range(2):
                        nc.tensor.matmul(
                            bc_t[64 * hp : 64 * hp + 64,
                                 m * 128 : (m + 1) * 128],
                            rcol8[:, m, 2 * hh + hp : 2 * hh + hp + 1]
                            .to_broadcast((128, 64)),
                            identf_sb, start=True, stop=True,
                        )
                nc.vector.tensor_tensor(
                    xT[:, hh, g * 512 : (g + 1) * 512], xus_l[hh], bc_t, OP.mult
                )

        def c_chunk(m):
            for half in range(2):
                ps_o = psS.tile([128, 512], f32, tag="pss")
                for p in range(4):
                    nc.tensor.matmul(
                        ps_o,
                        xT[:, p, m * 128 : (m + 1) * 128],
                        wout_sb[:, p, half * 512 : (half + 1) * 512],
                        start=(p == 0),
                        stop=(p == 3),
                    )
                ob = out_pool.tile([128, 512], f32, tag="ob")
                nc.vector.tensor_copy(ob, ps_o)
                nc.sync.dma_start(
                    out_d[m * 128 : (m + 1) * 128,
                          half * 512 : (half + 1) * 512],
                    ob,
                )

        pend = None
        for si in range(4):
            r = a_proj(si)
            if pend is not None:
                a_tail(*pend)
            pend = (si, r)
        a_tail(*pend)
        for g in range(NG):
            zsb8 = nrm_pool.tile([HL, 512], f32, tag="zsb8")
            xus_l = []
            pend = None
            for hh in range(4):
                xus_l.append(b_pair(g, hh, zsb8))
                if pend is not None:
                    a_tail(*pend)
                    pend = None
                if g < NG - 1:
                    si = 4 * (g + 1) + hh
                    pend = (si, a_proj(si))
                if g >= 1:
                    c_chunk(4 * (g - 1) + hh)
            if pend is not None:
                a_tail(*pend)
            b_norm(g, zsb8, xus_l)
        for m in range(4 * (NG - 1), ST):
            c_chunk(m)

    nc.compile()
    return nc


def _numpy_fallback(x, w_q, w_k, w_v, w_out, seg, mask):
    """Exact numpy replica of the reference for non-causal masks."""
    frac = (2.0 * np.arange(Dh // 2, dtype=np.float32)) / Dh
    ts = (MAX_WAVELENGTH ** frac).astype(np.float32)

    def rope(t, pos):
        sinu = pos.astype(np.float32)[:, :, None] / ts  # [B,S,32]
        sn, cs = np.sin(sinu), np.cos(sinu)
        sn, cs = sn[:, :, None, :], cs[:, :, None, :]
        f, s_ = t[..., :32], t[..., 32:]
        return np.concatenate([f * cs - s_ * sn, s_ * cs + f * sn], -1)

    q = np.einsum("bsd,dhk->bshk", x, w_q)
    k = np.einsum("bsd,dhk->bshk", x, w_k)
    v = np.einsum("bsd,dhk->bshk", x, w_v)
    q, k = rope(q, seg), rope(k, seg)
    q = q / np.sqrt(np.float32(Dh))
    attn = np.einsum("bqhd,bkhd->bhqk", q, k)
    attn = np.where(mask, attn, np.finfo(np.float32).min)
    attn = attn - attn.max(-1, keepdims=True)
    e = np.exp(attn)
    attn = e / e.sum(-1, keepdims=True)
    xo = np.einsum("bhqk,bkhd->bqhd", attn, v)
    return np.einsum("bqhd,hdm->bqm", xo, w_out).astype(np.float32)


def _host_inputs(x, w_q, w_k, w_v, w_out, seg):
    frac = (2.0 * np.arange(Dh // 2, dtype=np.float32)) / Dh
    ts = (MAX_WAVELENGTH ** frac).astype(np.float32)
    identb = np.eye(128, dtype=np.float32).astype(BF16)
    keept = np.triu(np.ones((128, 128), dtype=np.float32)).astype(BF16)
    identf = np.eye(128, dtype=np.float32)

    in_maps = []
    for core in range(NCORES):
        b, g = core // 2, core % 2
        hs = slice(g * HL, (g + 1) * HL)
        wq_s = (w_q[:, hs, :] / np.float32(np.sqrt(Dh))).reshape(D, HL * Dh)
        wk_s = w_k[:, hs, :].reshape(D, HL * Dh)
        wv_s = w_v[:, hs, :].reshape(D, HL * Dh)
        wqkv = np.ascontiguousarray(
            np.concatenate([wq_s, wk_s, wv_s], axis=1), dtype=np.float32
        ).astype(BF16)
        woutp = np.stack(
            [
                w_out[g * HL + 2 * p : g * HL + 2 * p + 2].reshape(128, D)
                for p in range(4)
            ]
        ).astype(BF16)
        sinu = seg[b].astype(np.float32)[:, None] / ts  # [S, 32]
        in_maps.append(
            {
                "xt": np.ascontiguousarray(x[b].T).astype(BF16),
                "wqkv": wqkv,
                "woutp": np.ascontiguousarray(woutp),
                "cost": np.cos(sinu).astype(BF16),
                "sint": np.sin(sinu).astype(BF16),
                "identb": identb,
                "keept": keept,
                "identf": identf,
            }
        )
    return in_maps


def _run(in_maps, trace=False):
    from concourse.bass_utils import run_bass_kernel_spmd

    if "nc" not in _CACHE:
        _CACHE["nc"] = _build_bass()
    return run_bass_kernel_spmd(
        _CACHE["nc"], in_maps, core_ids=list(range(NCORES)), trace=trace
    )


def kernel(**inputs):
    x = np.asarray(inputs["inputs"], dtype=np.float32)
    w_q = np.asarray(inputs["w_q"], dtype=np.float32)
    w_k = np.asarray(inputs["w_k"], dtype=np.float32)
    w_v = np.asarray(inputs["w_v"], dtype=np.float32)
    w_out = np.asarray(inputs["w_out"], dtype=np.float32)
    seg = np.asarray(inputs["segment_positions"])
    mask = np.asarray(inputs["mask"])

    causal = np.tril(np.ones((S, S), dtype=bool))
    if not all(np.array_equal(mask[b, 0], causal) for b in range(B)):
        return _numpy_fallback(x, w_q, w_k, w_v, w_out, seg, mask)

    in_maps = _host_inputs(x, w_q, w_k, w_v, w_out, seg)
    res = _run(in_maps)
    outs = [r_["out"] for r_ in res.results]
    result = np.empty((B, S, D), dtype=np.float32)
    for b in range(B):
        result[b] = outs[2 * b] + outs[2 * b + 1]
    return result


# revision 33
# speedup vs baseline: 1.1077x; 1.1077x over previous
"""Trainium2 Bass kernel for nn_AttentionBlock (B=4, S=2048, D=1024, H=16, Dh=64).

Sharding: 8 cores = 4 batches x 2 head-groups (8 heads each). Every core runs
the same Bass program on different input slices. The output projection is
row-sharded over head-groups, so the host sums the two partial outputs per
batch (the "all-reduce" of the sharding hint, done on host since we return
full outputs anyway).

Per-core pipeline (all matmuls bf16):
  A) QKV projection: lhsT = X^T chunks [128,128], rhs = Wqkv [128,1536 cols]
     -> psum [128(S-tile), 512] per q/k/v. The psum is copied to SBUF bf16
     (DVE), RoPE applied with bf16 2x-mode DVE ops (4 independent temps),
     then PE-transposed per head into qT/kT [Dh, S] packs. V goes to SBUF
     augmented with a ones column (V_aug [Sk,65]).
  B) Attention, two heads (one partition-half each) interleaved per k-tile:
     sT[Sk-tile 128, Sq 512] = kT_tile.T @ qT_group (alternating PE row
     groups so LDWEIGHTS hides). exp on ScalarE (PSUM->SBUF bf16). The
     causal mask is a [128,128] upper-tri multiply on DVE applied to the
     exp output of diagonal tiles only. AV: x_aug^T[65, Sq] += V_aug.T @ pT
     with row 64 accumulating the softmax denominator Z for free.
     Normalization: copy the two Z rows -> [2,512], reciprocal on DVE, one
     K=2 broadcast matmul -> [128,512] psum, single DVE multiply into xT.
  C) Output projection: out[Sq,512] += xT_pair.T @ WoutPair, DMA to HBM.

Issue order software-pipelines groups: QKV chunks of group g+1 and output
chunks of group g-1 are interleaved between attention head-pairs of group g
so the PE always has filler work while ScalarE runs exp.
"""

import sys

for _p in ("/opt/pypackages", "/opt/trn_rl_repo"):
    if _p not in sys.path:
        sys.path.insert(0, _p)

import numpy as np
import ml_dtypes

BF16 = ml_dtypes.bfloat16

B, S, D, H, Dh = 4, 2048, 1024, 16, 64
HL = H // 2          # heads per core
NCORES = 8
ST = S // 128        # 16 S-tiles of 128
NG = S // 512        # 4 q-groups of 512
MAX_WAVELENGTH = 10000.0

_CACHE = {}


def _build_bass():
    import concourse.bass as bass
    import concourse.mybir as mybir
    from concourse import bacc
    from concourse.tile import TileContext
    from contextlib import ExitStack

    f32 = mybir.dt.float32
    bf16 = mybir.dt.bfloat16
    AT = mybir.ActivationFunctionType
    OP = mybir.AluOpType

    nc = bacc.Bacc("TRN2", target_bir_lowering=False)

    xt_d = nc.dram_tensor("xt", [D, S], bf16, kind="ExternalInput")
    wqkv_d = nc.dram_tensor("wqkv", [D, 3 * HL * Dh], bf16, kind="ExternalInput")
    wout_d = nc.dram_tensor("woutp", [4, 128, D], bf16, kind="ExternalInput")
    cos_d = nc.dram_tensor("cost", [S, Dh // 2], bf16, kind="ExternalInput")
    sin_d = nc.dram_tensor("sint", [S, Dh // 2], bf16, kind="ExternalInput")
    identb_d = nc.dram_tensor("identb", [128, 128], bf16, kind="ExternalInput")
    keept_d = nc.dram_tensor("keept", [128, 128], bf16, kind="ExternalInput")
    identf_d = nc.dram_tensor("identf", [128, 128], f32, kind="ExternalInput")
    out_d = nc.dram_tensor("out", [S, D], f32, kind="ExternalOutput")

    with TileContext(nc) as tc, ExitStack() as ctx:
        consts = ctx.enter_context(tc.tile_pool(name="consts", bufs=1))
        persist = ctx.enter_context(tc.tile_pool(name="persist", bufs=1))

        # DMA order matters: what a_chunk(0) needs comes first so the PE can
        # start while the rest of the inputs stream in.
        identb_sb = consts.tile([128, 128], bf16, tag="identb")
        nc.sync.dma_start(identb_sb, identb_d[:, :])
        cos_sb = consts.tile([128, ST, 32], bf16, tag="cos")
        nc.sync.dma_start(cos_sb, cos_d.rearrange("(t p) f -> p t f", p=128))
        sin_sb = consts.tile([128, ST, 32], bf16, tag="sin")
        nc.sync.dma_start(sin_sb, sin_d.rearrange("(t p) f -> p t f", p=128))
        wq_sb = consts.tile([128, 8, 1536], bf16, tag="wqkv")
        wq_r = wqkv_d.rearrange("(c p) n -> p c n", p=128)
        xt_full = consts.tile([128, 8, S], bf16, tag="xtf")
        xt_r = xt_d.rearrange("(c p) s -> p c s", p=128)
        nc.sync.dma_start(wq_sb[:, :, 0:512], wq_r[:, :, 0:512])
        nc.sync.dma_start(
            xt_full[:, :, 0:512], xt_r[:, :, 0:512]
        )
        nc.sync.dma_start(wq_sb[:, :, 512:1024], wq_r[:, :, 512:1024])
        nc.sync.dma_start(wq_sb[:, :, 1024:1536], wq_r[:, :, 1024:1536])
        for sc in range(1, 4):
            nc.sync.dma_start(
                xt_full[:, :, sc * 512 : (sc + 1) * 512],
                xt_r[:, :, sc * 512 : (sc + 1) * 512],
            )
        keept_sb = consts.tile([128, 128], bf16, tag="keept")
        nc.sync.dma_start(keept_sb, keept_d[:, :])
        identf_sb = consts.tile([128, 128], f32, tag="identf")
        nc.sync.dma_start(identf_sb, identf_d[:, :])
        wout_sb = consts.tile([128, 4, 1024], bf16, tag="wout")
        nc.sync.dma_start(wout_sb, wout_d.rearrange("q p n -> p q n"))

        qT = persist.tile([128, 4, S], bf16, tag="qT")
        kT = persist.tile([128, 4, S], bf16, tag="kT")
        xT = persist.tile([128, 4, S], bf16, tag="xT")
        vaug = persist.tile([128, HL, ST, Dh + 1], bf16, tag="vaug")
        nc.scalar.activation(
            vaug[:, :, :, Dh : Dh + 1],
            identb_sb[:, 0:1, None].to_broadcast((128, HL, ST, 1)),
            AT.Identity, bias=1.0, scale=0.0,
        )

        rw_pool = ctx.enter_context(tc.tile_pool(name="ropew", bufs=3))
        pt_pool = ctx.enter_context(tc.tile_pool(name="ptp", bufs=6))
        nrm_pool = ctx.enter_context(tc.tile_pool(name="nrm", bufs=2))
        xus_pool = ctx.enter_context(tc.tile_pool(name="xus", bufs=5))
        out_pool = ctx.enter_context(tc.tile_pool(name="outp", bufs=3))
        psQ = ctx.enter_context(tc.tile_pool(name="psQ", bufs=2, space="PSUM"))
        psS = ctx.enter_context(tc.tile_pool(name="psS", bufs=2, space="PSUM"))
        psX = ctx.enter_context(tc.tile_pool(name="psX", bufs=2, space="PSUM"))

        def a_proj(si):
            # q/k projection + RoPE (DVE) + v projection.  The PE-transposes
            # of the rotated q/k are issued later (a_tail) so other PE work
            # can fill the RoPE latency.
            cos_b = cos_sb[:, si, None, :].to_broadcast((128, HL, 32))
            sin_b = sin_sb[:, si, None, :].to_broadcast((128, HL, 32))
            rots = []
            for qkv in (0, 1):
                ps = psQ.tile([128, 512], f32, tag="pqkv", name=f"ps{qkv}")
                for c in range(8):
                    nc.tensor.matmul(
                        ps, xt_full[:, c, si * 128 : (si + 1) * 128],
                        wq_sb[:, c, qkv * 512 : qkv * 512 + 512],
                        start=(c == 0), stop=(c == 7),
                    )
                xsb = rw_pool.tile([128, HL, Dh], bf16, tag="xsb")
                nc.vector.tensor_copy(xsb, ps.rearrange("p (h d) -> p h d", h=HL))
                x1, x2 = xsb[:, :, 0:32], xsb[:, :, 32:64]
                rot = rw_pool.tile([128, HL, Dh], bf16, tag="rot")
                t1 = rw_pool.tile([128, HL, 32], bf16, tag="t1")
                t2 = rw_pool.tile([128, HL, 32], bf16, tag="t2")
                t3 = rw_pool.tile([128, HL, 32], bf16, tag="t3")
                t4 = rw_pool.tile([128, HL, 32], bf16, tag="t4")
                nc.vector.tensor_tensor(t1, x1, cos_b, OP.mult)
                nc.vector.tensor_tensor(t2, x2, sin_b, OP.mult)
                nc.vector.tensor_tensor(t3, x1, sin_b, OP.mult)
                nc.vector.tensor_tensor(t4, x2, cos_b, OP.mult)
                nc.vector.tensor_tensor(rot[:, :, 0:32], t1, t2, OP.subtract)
                nc.vector.tensor_tensor(rot[:, :, 32:64], t3, t4, OP.add)
                rots.append(rot)
            ps_v = psQ.tile([128, 512], f32, tag="pqkv", name="ps_v")
            for c in range(8):
                nc.tensor.matmul(
                    ps_v, xt_full[:, c, si * 128 : (si + 1) * 128],
                    wq_sb[:, c, 1024:1536],
                    start=(c == 0), stop=(c == 7),
                )
            nc.vector.tensor_copy(
                vaug[:, :, si, 0:Dh],
                ps_v.rearrange("p (h d) -> p h d", h=HL),
            )
            return rots

        def a_tail(si, rots):
            for dstT, rot in zip((qT, kT), rots):
                rotf = rot.rearrange("p h d -> p (h d)")
                ps_t = psQ.tile([128, 512], f32, tag="pqkv", name="ps_t")
                for jj in range(4):
                    nc.tensor.matmul(
                        ps_t[:, jj * 128 : (jj + 1) * 128],
                        rotf[:, jj * 128 : (jj + 1) * 128],
                        identb_sb, start=True, stop=True,
                    )
                nc.vector.tensor_copy(
                    dstT[:, :, si * 128 : (si + 1) * 128],
                    ps_t.rearrange("p (j s) -> p j s", j=4),
                )

        def b_pair(g, hh, zsb8):
            nj = 4 * (g + 1)
            px = [
                psX.tile([Dh + 1, 512], f32, tag="psx", name=f"px{hp}")
                for hp in range(2)
            ]
            for jj in range(0, nj, 2):
                ss = []
                for j in (jj, jj + 1):
                    diag = j >= 4 * g
                    c0 = 128 * (j - 4 * g) if diag else 0
                    ps_s = psS.tile([128, 2, 512], f32, tag="pss")
                    for hp in range(2):
                        nc.tensor.matmul(
                            ps_s[:, hp, c0:512],
                            kT[64 * hp : 64 * hp + 64, hh,
                               j * 128 : (j + 1) * 128],
                            qT[64 * hp : 64 * hp + 64, hh,
                               g * 512 + c0 : (g + 1) * 512],
                            start=True, stop=True,
                        )
                    pt = pt_pool.tile([128, 2, 512], bf16, tag="pt")
                    nc.scalar.activation(pt[:, :, c0:512], ps_s[:, :, c0:512],
                                         AT.Exp)
                    if diag:
                        for hp in range(2):
                            nc.vector.tensor_tensor(
                                pt[:, hp, c0 : c0 + 128],
                                pt[:, hp, c0 : c0 + 128],
                                keept_sb, OP.mult,
                            )
                    ss.append((j, c0, pt))
                for j, c0, pt in ss:
                    for hp in range(2):
                        nc.tensor.matmul(
                            px[hp][:, c0:512],
                            vaug[:, 2 * hh + hp, j, :], pt[:, hp, c0:512],
                            start=(j == 0), stop=(j == nj - 1),
                        )
            # stash Z rows into zsb8 (partition placement needs the gpsimd DMA)
            # and the unnormalized x into xus; normalization happens at the
            # group tail once all 8 heads' Z are in place.
            zta = nrm_pool.tile([1, 512], f32, tag="zta")
            nc.vector.tensor_copy(zta, px[0][Dh : Dh + 1, :])
            ztb = nrm_pool.tile([1, 512], f32, tag="ztb")
            nc.vector.tensor_copy(ztb, px[1][Dh : Dh + 1, :])
            nc.gpsimd.dma_start(zsb8[2 * hh : 2 * hh + 1, :], zta)
            nc.gpsimd.dma_start(zsb8[2 * hh + 1 : 2 * hh + 2, :], ztb)
            xus = xus_pool.tile([128, 512], bf16, tag="xus", name=f"xus{hh}")
            nc.vector.tensor_copy(xus[0:64, :], px[0][0:Dh, :])
            nc.vector.tensor_copy(xus[64:128, :], px[1][0:Dh, :])
            return xus

        def b_norm(g, zsb8, xus_l):
            # Z [8,512] -> columns [128,(m,h)] via K=8 transpose matmuls ->
            # reciprocal on fat partitions -> per-pair broadcast matmuls with
            # a stride-0 (free-dim broadcast) lhsT -> one DVE multiply per pair
            zc_t = psX.tile([128, 512], f32, tag="psx", name="zc_t")
            zcv = zc_t[:, 0:32].rearrange("p (m h) -> p m h", m=4)
            for m in range(4):
                nc.tensor.matmul(
                    zcv[:, m, :], zsb8[:, m * 128 : (m + 1) * 128],
                    identf_sb[0:HL, 0:HL], start=True, stop=True,
                )
            rcol8 = nrm_pool.tile([128, 4, HL], f32, tag="rcol")
            nc.vector.reciprocal(rcol8, zcv)
            for hh in range(4):
                bc_t = psX.tile([128, 512], f32, tag="psx", name="bc_t")
                for m in range(4):
                    for hp in range(2):
                        nc.tensor.matmul(
                            bc_t[64 * hp : 64 * hp + 64,
                                 m * 128 : (m + 1) * 128],
                            rcol8[:, m, 2 * hh + hp : 2 * hh + hp + 1]
                            .to_broadcast((128, 64)),
                            identf_sb, start=True, stop=True,
                        )
                nc.vector.tensor_tensor(
                    xT[:, hh, g * 512 : (g + 1) * 512], xus_l[hh], bc_t, OP.mult
                )

        def c_chunk(m):
            for half in range(2):
                ps_o = psQ.tile([128, 512], f32, tag="pqkv", name="ps_o")
                for p in range(4):
                    nc.tensor.matmul(
                        ps_o,
                        xT[:, p, m * 128 : (m + 1) * 128],
                        wout_sb[:, p, half * 512 : (half + 1) * 512],
                        start=(p == 0),
                        stop=(p == 3),
                    )
                ob = out_pool.tile([128, 512], f32, tag="ob")
                nc.vector.tensor_copy(ob, ps_o)
                nc.sync.dma_start(
                    out_d[m * 128 : (m + 1) * 128,
                          half * 512 : (half + 1) * 512],
                    ob,
                )

        pend = None
        for si in range(4):
            r = a_proj(si)
            if pend is not None:
                a_tail(*pend)
            pend = (si, r)
        a_tail(*pend)
        for g in range(NG):
            zsb8 = nrm_pool.tile([HL, 512], f32, tag="zsb8")
            xus_l = []
            pend = None
            for hh in range(4):
                xus_l.append(b_pair(g, hh, zsb8))
                if pend is not None:
                    a_tail(*pend)
                    pend = None
                if g < NG - 1:
                    si = 4 * (g + 1) + hh
                    pend = (si, a_proj(si))
                if g >= 1:
                    c_chunk(4 * (g - 1) + hh)
            if pend is not None:
                a_tail(*pend)
            b_norm(g, zsb8, xus_l)
        for m in range(4 * (NG - 1), ST):
            c_chunk(m)

    nc.compile()
    return nc


def _numpy_fallback(x, w_q, w_k, w_v, w_out, seg, mask):
    """Exact numpy replica of the reference for non-causal masks."""
    frac = (2.0 * np.arange(Dh // 2, dtype=np.float32)) / Dh
    ts = (MAX_WAVELENGTH ** frac).astype(np.float32)

    def rope(t, pos):
        sinu = pos.astype(np.float32)[:, :, None] / ts  # [B,S,32]
        sn, cs = np.sin(sinu), np.cos(sinu)
        sn, cs = sn[:, :, None, :], cs[:, :, None, :]
        f, s_ = t[..., :32], t[..., 32:]
        return np.concatenate([f * cs - s_ * sn, s_ * cs + f * sn], -1)

    q = np.einsum("bsd,dhk->bshk", x, w_q)
    k = np.einsum("bsd,dhk->bshk", x, w_k)
    v = np.einsum("bsd,dhk->bshk", x, w_v)
    q, k = rope(q, seg), rope(k, seg)
    q = q / np.sqrt(np.float32(Dh))
    attn = np.einsum("bqhd,bkhd->bhqk", q, k)
    attn = np.where(mask, attn, np.finfo(np.float32).min)
    attn = attn - attn.max(-1, keepdims=True)
    e = np.exp(attn)
    attn = e / e.sum(-1, keepdims=True)
    xo = np.einsum("bhqk,bkhd->bqhd", attn, v)
    return np.einsum("bqhd,hdm->bqm", xo, w_out).astype(np.float32)


def _host_inputs(x, w_q, w_k, w_v, w_out, seg):
    frac = (2.0 * np.arange(Dh // 2, dtype=np.float32)) / Dh
    ts = (MAX_WAVELENGTH ** frac).astype(np.float32)
    identb = np.eye(128, dtype=np.float32).astype(BF16)
    keept = np.triu(np.ones((128, 128), dtype=np.float32)).astype(BF16)
    identf = np.eye(128, dtype=np.float32)

    in_maps = []
    for core in range(NCORES):
        b, g = core // 2, core % 2
        hs = slice(g * HL, (g + 1) * HL)
        wq_s = (w_q[:, hs, :] / np.float32(np.sqrt(Dh))).reshape(D, HL * Dh)
        wk_s = w_k[:, hs, :].reshape(D, HL * Dh)
        wv_s = w_v[:, hs, :].reshape(D, HL * Dh)
        wqkv = np.ascontiguousarray(
            np.concatenate([wq_s, wk_s, wv_s], axis=1), dtype=np.float32
        ).astype(BF16)
        woutp = np.stack(
            [
                w_out[g * HL + 2 * p : g * HL + 2 * p + 2].reshape(128, D)
                for p in range(4)
            ]
        ).astype(BF16)
        sinu = seg[b].astype(np.float32)[:, None] / ts  # [S, 32]
        in_maps.append(
            {
                "xt": np.ascontiguousarray(x[b].T).astype(BF16),
                "wqkv": wqkv,
                "woutp": np.ascontiguousarray(woutp),
                "cost": np.cos(sinu).astype(BF16),
                "sint": np.sin(sinu).astype(BF16),
                "identb": identb,
                "keept": keept,
                "identf": identf,
            }
        )
    return in_maps


def _run(in_maps, trace=False):
    from concourse.bass_utils import run_bass_kernel_spmd

    if "nc" not in _CACHE:
        _CACHE["nc"] = _build_bass()
    return run_bass_kernel_spmd(
        _CACHE["nc"], in_maps, core_ids=list(range(NCORES)), trace=trace
    )


def kernel(**inputs):
    x = np.asarray(inputs["inputs"], dtype=np.float32)
    w_q = np.asarray(inputs["w_q"], dtype=np.float32)
    w_k = np.asarray(inputs["w_k"], dtype=np.float32)
    w_v = np.asarray(inputs["w_v"], dtype=np.float32)
    w_out = np.asarray(inputs["w_out"], dtype=np.float32)
    seg = np.asarray(inputs["segment_positions"])
    mask = np.asarray(inputs["mask"])

    causal = np.tril(np.ones((S, S), dtype=bool))
    if not all(np.array_equal(mask[b, 0], causal) for b in range(B)):
        return _numpy_fallback(x, w_q, w_k, w_v, w_out, seg, mask)

    in_maps = _host_inputs(x, w_q, w_k, w_v, w_out, seg)
    res = _run(in_maps)
    outs = [r_["out"] for r_ in res.results]
    result = np.empty((B, S, D), dtype=np.float32)
    for b in range(B):
        result[b] = outs[2 * b] + outs[2 * b + 1]
    return result


# revision 38
# speedup vs baseline: 1.1317x; 1.0217x over previous
"""Trainium2 Bass kernel for nn_AttentionBlock (B=4, S=2048, D=1024, H=16, Dh=64).

Sharding: 8 cores = 4 batches x 2 head-groups (8 heads each). Every core runs
the same Bass program on different input slices. The output projection is
row-sharded over head-groups, so the host sums the two partial outputs per
batch (the "all-reduce" of the sharding hint, done on host since we return
full outputs anyway).

Per-core pipeline (all matmuls bf16):
  A) QKV projection: lhsT = X^T chunks [128,128], rhs = Wqkv [128,1536 cols]
     -> psum [128(S-tile), 512] per q/k/v. The psum is copied to SBUF bf16
     (DVE), RoPE applied with bf16 2x-mode DVE ops (4 independent temps),
     then PE-transposed per head into qT/kT [Dh, S] packs. V goes to SBUF
     augmented with a ones column (V_aug [Sk,65]).
  B) Attention, two heads (one partition-half each) interleaved per k-tile:
     sT[Sk-tile 128, Sq 512] = kT_tile.T @ qT_group (alternating PE row
     groups so LDWEIGHTS hides). exp on ScalarE (PSUM->SBUF bf16). The
     causal mask is a [128,128] upper-tri multiply on DVE applied to the
     exp output of diagonal tiles only. AV: x_aug^T[65, Sq] += V_aug.T @ pT
     with row 64 accumulating the softmax denominator Z for free.
     Normalization: copy the two Z rows -> [2,512], reciprocal on DVE, one
     K=2 broadcast matmul -> [128,512] psum, single DVE multiply into xT.
  C) Output projection: out[Sq,512] += xT_pair.T @ WoutPair, DMA to HBM.

Issue order software-pipelines groups: QKV chunks of group g+1 and output
chunks of group g-1 are interleaved between attention head-pairs of group g
so the PE always has filler work while ScalarE runs exp.
"""

import sys

for _p in ("/opt/pypackages", "/opt/trn_rl_repo"):
    if _p not in sys.path:
        sys.path.insert(0, _p)

import numpy as np
import ml_dtypes

BF16 = ml_dtypes.bfloat16

B, S, D, H, Dh = 4, 2048, 1024, 16, 64
HL = H // 2          # heads per core
NCORES = 8
ST = S // 128        # 16 S-tiles of 128
NG = S // 512        # 4 q-groups of 512
MAX_WAVELENGTH = 10000.0

_CACHE = {}


def _build_bass():
    import concourse.bass as bass
    import concourse.mybir as mybir
    from concourse import bacc
    from concourse.tile import TileContext
    from contextlib import ExitStack

    f32 = mybir.dt.float32
    bf16 = mybir.dt.bfloat16
    AT = mybir.ActivationFunctionType
    OP = mybir.AluOpType

    nc = bacc.Bacc("TRN2", target_bir_lowering=False)

    # All inputs arrive pre-arranged on the host into the exact SBUF layouts
    # (partition-major, contiguous per partition) so every input DMA moves
    # large contiguous runs instead of 1KB gather descriptors.
    xt_d = nc.dram_tensor("xt", [128, 4, 8, 512], bf16, kind="ExternalInput")
    wqkv_d = nc.dram_tensor("wqkv", [128, 3, 8, 512], bf16, kind="ExternalInput")
    wout_d = nc.dram_tensor("woutp", [128, 4, D], bf16, kind="ExternalInput")
    cos_d = nc.dram_tensor("cost", [128, ST, Dh // 2], bf16, kind="ExternalInput")
    sin_d = nc.dram_tensor("sint", [128, ST, Dh // 2], bf16, kind="ExternalInput")
    identb_d = nc.dram_tensor("identb", [128, 128], bf16, kind="ExternalInput")
    keept_d = nc.dram_tensor("keept", [128, 128], bf16, kind="ExternalInput")
    identf_d = nc.dram_tensor("identf", [128, 128], f32, kind="ExternalInput")
    out_d = nc.dram_tensor("out", [S, D], f32, kind="ExternalOutput")

    with TileContext(nc) as tc, ExitStack() as ctx:
        consts = ctx.enter_context(tc.tile_pool(name="consts", bufs=1))
        persist = ctx.enter_context(tc.tile_pool(name="persist", bufs=1))

        # DMA order matters: what a_chunk(0) needs comes first so the PE can
        # start while the rest of the inputs stream in.
        identb_sb = consts.tile([128, 128], bf16, tag="identb")
        nc.sync.dma_start(identb_sb, identb_d[:, :])
        cos_sb = consts.tile([128, ST, 32], bf16, tag="cos")
        nc.sync.dma_start(cos_sb, cos_d[:, :, :])
        sin_sb = consts.tile([128, ST, 32], bf16, tag="sin")
        nc.sync.dma_start(sin_sb, sin_d[:, :, :])
        wq_sb = consts.tile([128, 3, 8, 512], bf16, tag="wqkv")
        xt_full = consts.tile([128, 4, 8, 512], bf16, tag="xtf")
        nc.sync.dma_start(wq_sb[:, 0, :, :], wqkv_d[:, 0, :, :])
        nc.sync.dma_start(xt_full[:, 0, :, :], xt_d[:, 0, :, :])
        nc.sync.dma_start(wq_sb[:, 1, :, :], wqkv_d[:, 1, :, :])
        nc.sync.dma_start(wq_sb[:, 2, :, :], wqkv_d[:, 2, :, :])
        for sc in range(1, 4):
            nc.sync.dma_start(xt_full[:, sc, :, :], xt_d[:, sc, :, :])
        keept_sb = consts.tile([128, 128], bf16, tag="keept")
        nc.sync.dma_start(keept_sb, keept_d[:, :])
        identf_sb = consts.tile([128, 128], f32, tag="identf")
        nc.sync.dma_start(identf_sb, identf_d[:, :])
        wout_sb = consts.tile([128, 4, 1024], bf16, tag="wout")
        nc.sync.dma_start(wout_sb, wout_d[:, :, :])

        qT = persist.tile([128, 4, S], bf16, tag="qT")
        kT = persist.tile([128, 4, S], bf16, tag="kT")
        xT = persist.tile([128, 4, S], bf16, tag="xT")
        vaug = persist.tile([128, HL, ST, Dh + 1], bf16, tag="vaug")
        nc.scalar.activation(
            vaug[:, :, :, Dh : Dh + 1],
            identb_sb[:, 0:1, None].to_broadcast((128, HL, ST, 1)),
            AT.Identity, bias=1.0, scale=0.0,
        )

        rw_pool = ctx.enter_context(tc.tile_pool(name="ropew", bufs=3))
        pt_pool = ctx.enter_context(tc.tile_pool(name="ptp", bufs=6))
        nrm_pool = ctx.enter_context(tc.tile_pool(name="nrm", bufs=2))
        xus_pool = ctx.enter_context(tc.tile_pool(name="xus", bufs=5))
        out_pool = ctx.enter_context(tc.tile_pool(name="outp", bufs=3))
        psQ = ctx.enter_context(tc.tile_pool(name="psQ", bufs=2, space="PSUM"))
        psS = ctx.enter_context(tc.tile_pool(name="psS", bufs=2, space="PSUM"))
        psX = ctx.enter_context(tc.tile_pool(name="psX", bufs=2, space="PSUM"))

        def a_proj(si):
            # q/k projection + RoPE (DVE) + v projection.  The PE-transposes
            # of the rotated q/k are issued later (a_tail) so other PE work
            # can fill the RoPE latency.
            cos_b = cos_sb[:, si, None, :].to_broadcast((128, HL, 32))
            sin_b = sin_sb[:, si, None, :].to_broadcast((128, HL, 32))
            rots = []
            for qkv in (0, 1):
                ps = psQ.tile([128, 512], f32, tag="pqkv", name=f"ps{qkv}")
                for c in range(8):
                    nc.tensor.matmul(
                        ps,
                        xt_full[:, si // 4, c,
                                (si % 4) * 128 : (si % 4) * 128 + 128],
                        wq_sb[:, qkv, c, :],
                        start=(c == 0), stop=(c == 7),
                    )
                xsb = rw_pool.tile([128, HL, Dh], bf16, tag="xsb")
                nc.vector.tensor_copy(xsb, ps.rearrange("p (h d) -> p h d", h=HL))
                x1, x2 = xsb[:, :, 0:32], xsb[:, :, 32:64]
                rot = rw_pool.tile([128, HL, Dh], bf16, tag="rot")
                t1 = rw_pool.tile([128, HL, 32], bf16, tag="t1")
                t2 = rw_pool.tile([128, HL, 32], bf16, tag="t2")
                t3 = rw_pool.tile([128, HL, 32], bf16, tag="t3")
                t4 = rw_pool.tile([128, HL, 32], bf16, tag="t4")
                nc.vector.tensor_tensor(t1, x1, cos_b, OP.mult)
                nc.vector.tensor_tensor(t2, x2, sin_b, OP.mult)
                nc.vector.tensor_tensor(t3, x1, sin_b, OP.mult)
                nc.vector.tensor_tensor(t4, x2, cos_b, OP.mult)
                nc.vector.tensor_tensor(rot[:, :, 0:32], t1, t2, OP.subtract)
                nc.vector.tensor_tensor(rot[:, :, 32:64], t3, t4, OP.add)
                rots.append(rot)
            ps_v = psQ.tile([128, 512], f32, tag="pqkv", name="ps_v")
            for c in range(8):
                nc.tensor.matmul(
                    ps_v,
                    xt_full[:, si // 4, c,
                            (si % 4) * 128 : (si % 4) * 128 + 128],
                    wq_sb[:, 2, c, :],
                    start=(c == 0), stop=(c == 7),
                )
            nc.vector.tensor_copy(
                vaug[:, :, si, 0:Dh],
                ps_v.rearrange("p (h d) -> p h d", h=HL),
            )
            return rots

        def a_tail(si, rots):
            for dstT, rot in zip((qT, kT), rots):
                rotf = rot.rearrange("p h d -> p (h d)")
                ps_t = psQ.tile([128, 512], f32, tag="pqkv", name="ps_t")
                for jj in range(4):
                    nc.tensor.matmul(
                        ps_t[:, jj * 128 : (jj + 1) * 128],
                        rotf[:, jj * 128 : (jj + 1) * 128],
                        identb_sb, start=True, stop=True,
                    )
                nc.vector.tensor_copy(
                    dstT[:, :, si * 128 : (si + 1) * 128],
                    ps_t.rearrange("p (j s) -> p j s", j=4),
                )

        def b_pair(g, hh, zsb8):
            nj = 4 * (g + 1)
            px = [
                psX.tile([Dh + 1, 512], f32, tag="psx", name=f"px{hp}")
                for hp in range(2)
            ]
            for jj in range(0, nj, 2):
                ss = []
                for j in (jj, jj + 1):
                    diag = j >= 4 * g
                    c0 = 128 * (j - 4 * g) if diag else 0
                    ps_s = psS.tile([128, 2, 512], f32, tag="pss")
                    for hp in range(2):
                        nc.tensor.matmul(
                            ps_s[:, hp, c0:512],
                            kT[64 * hp : 64 * hp + 64, hh,
                               j * 128 : (j + 1) * 128],
                            qT[64 * hp : 64 * hp + 64, hh,
                               g * 512 + c0 : (g + 1) * 512],
                            start=True, stop=True,
                        )
                    pt = pt_pool.tile([128, 2, 512], bf16, tag="pt")
                    nc.scalar.activation(pt[:, :, c0:512], ps_s[:, :, c0:512],
                                         AT.Exp)
                    if diag:
                        for hp in range(2):
                            nc.vector.tensor_tensor(
                                pt[:, hp, c0 : c0 + 128],
                                pt[:, hp, c0 : c0 + 128],
                                keept_sb, OP.mult,
                            )
                    ss.append((j, c0, pt))
                for j, c0, pt in ss:
                    for hp in range(2):
                        nc.tensor.matmul(
                            px[hp][:, c0:512],
                            vaug[:, 2 * hh + hp, j, :], pt[:, hp, c0:512],
                            start=(j == 0), stop=(j == nj - 1),
                        )
            # stash Z rows into zsb8 (partition placement needs the gpsimd DMA)
            # and the unnormalized x into xus; normalization happens at the
            # group tail once all 8 heads' Z are in place.
            zta = nrm_pool.tile([1, 512], f32, tag="zta")
            nc.vector.tensor_copy(zta, px[0][Dh : Dh + 1, :])
            ztb = nrm_pool.tile([1, 512], f32, tag="ztb")
            nc.vector.tensor_copy(ztb, px[1][Dh : Dh + 1, :])
            nc.gpsimd.dma_start(zsb8[2 * hh : 2 * hh + 1, :], zta)
            nc.gpsimd.dma_start(zsb8[2 * hh + 1 : 2 * hh + 2, :], ztb)
            xus = xus_pool.tile([128, 512], bf16, tag="xus", name=f"xus{hh}")
            nc.vector.tensor_copy(xus[0:64, :], px[0][0:Dh, :])
            nc.vector.tensor_copy(xus[64:128, :], px[1][0:Dh, :])
            return xus

        def b_norm(g, zsb8, xus_l):
            # Z [8,512] -> columns [128,(m,h)] via K=8 transpose matmuls ->
            # reciprocal on fat partitions -> per-pair broadcast matmuls with
            # a stride-0 (free-dim broadcast) lhsT -> one DVE multiply per pair
            zc_t = psX.tile([128, 512], f32, tag="psx", name="zc_t")
            zcv = zc_t[:, 0:32].rearrange("p (m h) -> p m h", m=4)
            for m in range(4):
                nc.tensor.matmul(
                    zcv[:, m, :], zsb8[:, m * 128 : (m + 1) * 128],
                    identf_sb[0:HL, 0:HL], start=True, stop=True,
                )
            rcol8 = nrm_pool.tile([128, 4, HL], f32, tag="rcol")
            nc.vector.reciprocal(rcol8, zcv)
            for hh in range(4):
                bc_t = psX.tile([128, 512], f32, tag="psx", name="bc_t")
                for m in range(4):
                    for hp in range(2):
                        nc.tensor.matmul(
                            bc_t[64 * hp : 64 * hp + 64,
                                 m * 128 : (m + 1) * 128],
                            rcol8[:, m, 2 * hh + hp : 2 * hh + hp + 1]
                            .to_broadcast((128, 64)),
                            identf_sb, start=True, stop=True,
                        )
                nc.vector.tensor_tensor(
                    xT[:, hh, g * 512 : (g + 1) * 512], xus_l[hh], bc_t, OP.mult
                )

        def c_chunk(m):
            for half in range(2):
                ps_o = psQ.tile([128, 512], f32, tag="pqkv", name="ps_o")
                for p in range(4):
                    nc.tensor.matmul(
                        ps_o,
                        xT[:, p, m * 128 : (m + 1) * 128],
                        wout_sb[:, p, half * 512 : (half + 1) * 512],
                        start=(p == 0),
                        stop=(p == 3),
                    )
                ob = out_pool.tile([128, 512], f32, tag="ob")
                nc.vector.tensor_copy(ob, ps_o)
                nc.sync.dma_start(
                    out_d[m * 128 : (m + 1) * 128,
                          half * 512 : (half + 1) * 512],
                    ob,
                )

        pend = None
        for si in range(4):
            r = a_proj(si)
            if pend is not None:
                a_tail(*pend)
            pend = (si, r)
        a_tail(*pend)
        for g in range(NG):
            zsb8 = nrm_pool.tile([HL, 512], f32, tag="zsb8")
            xus_l = []
            pend = None
            for hh in range(4):
                xus_l.append(b_pair(g, hh, zsb8))
                if pend is not None:
                    a_tail(*pend)
                    pend = None
                if g < NG - 1:
                    si = 4 * (g + 1) + hh
                    pend = (si, a_proj(si))
                if g >= 1:
                    c_chunk(4 * (g - 1) + hh)
            if pend is not None:
                a_tail(*pend)
            b_norm(g, zsb8, xus_l)
        for m in range(4 * (NG - 1), ST):
            c_chunk(m)

    nc.compile()
    return nc


def _numpy_fallback(x, w_q, w_k, w_v, w_out, seg, mask):
    """Exact numpy replica of the reference for non-causal masks."""
    frac = (2.0 * np.arange(Dh // 2, dtype=np.float32)) / Dh
    ts = (MAX_WAVELENGTH ** frac).astype(np.float32)

    def rope(t, pos):
        sinu = pos.astype(np.float32)[:, :, None] / ts  # [B,S,32]
        sn, cs = np.sin(sinu), np.cos(sinu)
        sn, cs = sn[:, :, None, :], cs[:, :, None, :]
        f, s_ = t[..., :32], t[..., 32:]
        return np.concatenate([f * cs - s_ * sn, s_ * cs + f * sn], -1)

    q = np.einsum("bsd,dhk->bshk", x, w_q)
    k = np.einsum("bsd,dhk->bshk", x, w_k)
    v = np.einsum("bsd,dhk->bshk", x, w_v)
    q, k = rope(q, seg), rope(k, seg)
    q = q / np.sqrt(np.float32(Dh))
    attn = np.einsum("bqhd,bkhd->bhqk", q, k)
    attn = np.where(mask, attn, np.finfo(np.float32).min)
    attn = attn - attn.max(-1, keepdims=True)
    e = np.exp(attn)
    attn = e / e.sum(-1, keepdims=True)
    xo = np.einsum("bhqk,bkhd->bqhd", attn, v)
    return np.einsum("bqhd,hdm->bqm", xo, w_out).astype(np.float32)


def _host_inputs(x, w_q, w_k, w_v, w_out, seg):
    frac = (2.0 * np.arange(Dh // 2, dtype=np.float32)) / Dh
    ts = (MAX_WAVELENGTH ** frac).astype(np.float32)
    identb = np.eye(128, dtype=np.float32).astype(BF16)
    keept = np.triu(np.ones((128, 128), dtype=np.float32)).astype(BF16)
    identf = np.eye(128, dtype=np.float32)

    in_maps = []
    for core in range(NCORES):
        b, g = core // 2, core % 2
        hs = slice(g * HL, (g + 1) * HL)
        wq_s = (w_q[:, hs, :] / np.float32(np.sqrt(Dh))).reshape(D, HL * Dh)
        wk_s = w_k[:, hs, :].reshape(D, HL * Dh)
        wv_s = w_v[:, hs, :].reshape(D, HL * Dh)
        wqkv = np.ascontiguousarray(
            np.concatenate([wq_s, wk_s, wv_s], axis=1), dtype=np.float32
        ).astype(BF16)
        woutp = np.stack(
            [
                w_out[g * HL + 2 * p : g * HL + 2 * p + 2].reshape(128, D)
                for p in range(4)
            ]
        ).astype(BF16)
        sinu = seg[b].astype(np.float32)[:, None] / ts  # [S, 32]
        # pre-arrange into partition-major SBUF layouts (contiguous DMAs)
        xt4 = (
            x[b].T.reshape(8, 128, 4, 512).transpose(1, 2, 0, 3)
        )  # [p, sc, c, s]
        wq4 = wqkv.astype(np.float32).reshape(8, 128, 3, 512).transpose(
            1, 2, 0, 3
        )  # [p, qkv, c, n]
        wo3 = woutp.transpose(1, 0, 2)  # [p, q, n]
        cos3 = np.cos(sinu).reshape(ST, 128, 32).transpose(1, 0, 2)
        sin3 = np.sin(sinu).reshape(ST, 128, 32).transpose(1, 0, 2)
        in_maps.append(
            {
                "xt": np.ascontiguousarray(xt4).astype(BF16),
                "wqkv": np.ascontiguousarray(wq4).astype(BF16),
                "woutp": np.ascontiguousarray(wo3).astype(BF16),
                "cost": np.ascontiguousarray(cos3).astype(BF16),
                "sint": np.ascontiguousarray(sin3).astype(BF16),
                "identb": identb,
                "keept": keept,
                "identf": identf,
            }
        )
    return in_maps


def _run(in_maps, trace=False):
    from concourse.bass_utils import run_bass_kernel_spmd

    if "nc" not in _CACHE:
        _CACHE["nc"] = _build_bass()
    return run_bass_kernel_spmd(
        _CACHE["nc"], in_maps, core_ids=list(range(NCORES)), trace=trace
    )


def kernel(**inputs):
    x = np.asarray(inputs["inputs"], dtype=np.float32)
    w_q = np.asarray(inputs["w_q"], dtype=np.float32)
    w_k = np.asarray(inputs["w_k"], dtype=np.float32)
    w_v = np.asarray(inputs["w_v"], dtype=np.float32)
    w_out = np.asarray(inputs["w_out"], dtype=np.float32)
    seg = np.asarray(inputs["segment_positions"])
    mask = np.asarray(inputs["mask"])

    causal = np.tril(np.ones((S, S), dtype=bool))
    if not all(np.array_equal(mask[b, 0], causal) for b in range(B)):
        return _numpy_fallback(x, w_q, w_k, w_v, w_out, seg, mask)

    in_maps = _host_inputs(x, w_q, w_k, w_v, w_out, seg)
    res = _run(in_maps)
    outs = [r_["out"] for r_ in res.results]
    result = np.empty((B, S, D), dtype=np.float32)
    for b in range(B):
        result[b] = outs[2 * b] + outs[2 * b + 1]
    return result


# revision 41
# speedup vs baseline: 1.1361x; 1.0039x over previous
"""Trainium2 Bass kernel for nn_AttentionBlock (B=4, S=2048, D=1024, H=16, Dh=64).

Sharding: 8 cores = 4 batches x 2 head-groups (8 heads each). Every core runs
the same Bass program on different input slices. The output projection is
row-sharded over head-groups, so the host sums the two partial outputs per
batch (the "all-reduce" of the sharding hint, done on host since we return
full outputs anyway).

Per-core pipeline (all matmuls bf16):
  A) QKV projection: lhsT = X^T chunks [128,128], rhs = Wqkv [128,1536 cols]
     -> psum [128(S-tile), 512] per q/k/v. The psum is copied to SBUF bf16
     (DVE), RoPE applied with bf16 2x-mode DVE ops (4 independent temps),
     then PE-transposed per head into qT/kT [Dh, S] packs. V goes to SBUF
     augmented with a ones column (V_aug [Sk,65]).
  B) Attention, two heads (one partition-half each) interleaved per k-tile:
     sT[Sk-tile 128, Sq 512] = kT_tile.T @ qT_group (alternating PE row
     groups so LDWEIGHTS hides). exp on ScalarE (PSUM->SBUF bf16). The
     causal mask is a [128,128] upper-tri multiply on DVE applied to the
     exp output of diagonal tiles only. AV: x_aug^T[65, Sq] += V_aug.T @ pT
     with row 64 accumulating the softmax denominator Z for free.
     Normalization: copy the two Z rows -> [2,512], reciprocal on DVE, one
     K=2 broadcast matmul -> [128,512] psum, single DVE multiply into xT.
  C) Output projection: out[Sq,512] += xT_pair.T @ WoutPair, DMA to HBM.

Issue order software-pipelines groups: QKV chunks of group g+1 and output
chunks of group g-1 are interleaved between attention head-pairs of group g
so the PE always has filler work while ScalarE runs exp.
"""

import sys

for _p in ("/opt/pypackages", "/opt/trn_rl_repo"):
    if _p not in sys.path:
        sys.path.insert(0, _p)

import numpy as np
import ml_dtypes

BF16 = ml_dtypes.bfloat16

B, S, D, H, Dh = 4, 2048, 1024, 16, 64
HL = H // 2          # heads per core
NCORES = 8
ST = S // 128        # 16 S-tiles of 128
NG = S // 512        # 4 q-groups of 512
MAX_WAVELENGTH = 10000.0

_CACHE = {}


def _build_bass():
    import concourse.bass as bass
    import concourse.mybir as mybir
    from concourse import bacc
    from concourse.tile import TileContext
    from contextlib import ExitStack

    f32 = mybir.dt.float32
    bf16 = mybir.dt.bfloat16
    AT = mybir.ActivationFunctionType
    OP = mybir.AluOpType

    nc = bacc.Bacc("TRN2", target_bir_lowering=False)

    # All inputs arrive pre-arranged on the host into the exact SBUF layouts
    # (partition-major, contiguous per partition) so every input DMA moves
    # large contiguous runs instead of 1KB gather descriptors.
    xt_d = nc.dram_tensor("xt", [128, 4, 8, 512], bf16, kind="ExternalInput")
    wqkv_d = nc.dram_tensor("wqkv", [128, 3, 8, 512], bf16, kind="ExternalInput")
    wout_d = nc.dram_tensor("woutp", [128, 4, D], bf16, kind="ExternalInput")
    cos_d = nc.dram_tensor("cost", [128, ST, Dh // 2], bf16, kind="ExternalInput")
    sin_d = nc.dram_tensor("sint", [128, ST, Dh // 2], bf16, kind="ExternalInput")
    identb_d = nc.dram_tensor("identb", [128, 128], bf16, kind="ExternalInput")
    keept_d = nc.dram_tensor("keept", [128, 128], bf16, kind="ExternalInput")
    identf_d = nc.dram_tensor("identf", [128, 128], f32, kind="ExternalInput")
    out_d = nc.dram_tensor("out", [S, D], f32, kind="ExternalOutput")

    with TileContext(nc) as tc, ExitStack() as ctx:
        consts = ctx.enter_context(tc.tile_pool(name="consts", bufs=1))
        persist = ctx.enter_context(tc.tile_pool(name="persist", bufs=1))

        # DMA order matters: what a_chunk(0) needs comes first so the PE can
        # start while the rest of the inputs stream in.
        identb_sb = consts.tile([128, 128], bf16, tag="identb")
        nc.sync.dma_start(identb_sb, identb_d[:, :])
        cos_sb = consts.tile([128, ST, 32], bf16, tag="cos")
        nc.sync.dma_start(cos_sb, cos_d[:, :, :])
        sin_sb = consts.tile([128, ST, 32], bf16, tag="sin")
        nc.sync.dma_start(sin_sb, sin_d[:, :, :])
        wq_sb = consts.tile([128, 3, 8, 512], bf16, tag="wqkv")
        xt_full = consts.tile([128, 4, 8, 512], bf16, tag="xtf")
        nc.sync.dma_start(wq_sb[:, 0, :, :], wqkv_d[:, 0, :, :])
        nc.sync.dma_start(xt_full[:, 0, :, :], xt_d[:, 0, :, :])
        nc.sync.dma_start(wq_sb[:, 1, :, :], wqkv_d[:, 1, :, :])
        nc.sync.dma_start(wq_sb[:, 2, :, :], wqkv_d[:, 2, :, :])
        for sc in range(1, 4):
            nc.sync.dma_start(xt_full[:, sc, :, :], xt_d[:, sc, :, :])
        keept_sb = consts.tile([128, 128], bf16, tag="keept")
        nc.sync.dma_start(keept_sb, keept_d[:, :])
        identf_sb = consts.tile([128, 128], f32, tag="identf")
        nc.sync.dma_start(identf_sb, identf_d[:, :])
        wout_sb = consts.tile([128, 4, 1024], bf16, tag="wout")
        nc.sync.dma_start(wout_sb, wout_d[:, :, :])

        qT = persist.tile([128, 4, S], bf16, tag="qT")
        kT = persist.tile([128, 4, S], bf16, tag="kT")
        xT = persist.tile([128, 4, S], bf16, tag="xT")
        vaug = persist.tile([128, HL, ST, Dh + 1], bf16, tag="vaug")
        nc.scalar.activation(
            vaug[:, :, :, Dh : Dh + 1],
            identb_sb[:, 0:1, None].to_broadcast((128, HL, ST, 1)),
            AT.Identity, bias=1.0, scale=0.0,
        )

        rw_pool = ctx.enter_context(tc.tile_pool(name="ropew", bufs=3))
        pt_pool = ctx.enter_context(tc.tile_pool(name="ptp", bufs=6))
        nrm_pool = ctx.enter_context(tc.tile_pool(name="nrm", bufs=2))
        xus_pool = ctx.enter_context(tc.tile_pool(name="xus", bufs=5))
        out_pool = ctx.enter_context(tc.tile_pool(name="outp", bufs=3))
        psQ = ctx.enter_context(tc.tile_pool(name="psQ", bufs=2, space="PSUM"))
        psS = ctx.enter_context(tc.tile_pool(name="psS", bufs=2, space="PSUM"))
        psX = ctx.enter_context(tc.tile_pool(name="psX", bufs=2, space="PSUM"))

        def a_proj(si):
            # q/k projection + RoPE (DVE) + v projection.  The PE-transposes
            # of the rotated q/k are issued later (a_tail) so other PE work
            # can fill the RoPE latency.
            cos_b = cos_sb[:, si, None, :].to_broadcast((128, HL, 32))
            sin_b = sin_sb[:, si, None, :].to_broadcast((128, HL, 32))
            rots = []
            for qkv in (0, 1):
                ps = psQ.tile([128, 512], f32, tag="pqkv", name=f"ps{qkv}")
                for c in range(8):
                    nc.tensor.matmul(
                        ps,
                        xt_full[:, si // 4, c,
                                (si % 4) * 128 : (si % 4) * 128 + 128],
                        wq_sb[:, qkv, c, :],
                        start=(c == 0), stop=(c == 7),
                    )
                xsb = rw_pool.tile([128, HL, Dh], bf16, tag="xsb")
                if si < 4:
                    # prologue: ScalarE is idle until the first exp
                    nc.scalar.copy(xsb, ps.rearrange("p (h d) -> p h d", h=HL))
                else:
                    nc.vector.tensor_copy(
                        xsb, ps.rearrange("p (h d) -> p h d", h=HL)
                    )
                x1, x2 = xsb[:, :, 0:32], xsb[:, :, 32:64]
                rot = rw_pool.tile([128, HL, Dh], bf16, tag="rot")
                t1 = rw_pool.tile([128, HL, 32], bf16, tag="t1")
                t2 = rw_pool.tile([128, HL, 32], bf16, tag="t2")
                t3 = rw_pool.tile([128, HL, 32], bf16, tag="t3")
                t4 = rw_pool.tile([128, HL, 32], bf16, tag="t4")
                nc.vector.tensor_tensor(t1, x1, cos_b, OP.mult)
                nc.vector.tensor_tensor(t2, x2, sin_b, OP.mult)
                nc.vector.tensor_tensor(t3, x1, sin_b, OP.mult)
                nc.vector.tensor_tensor(t4, x2, cos_b, OP.mult)
                nc.vector.tensor_tensor(rot[:, :, 0:32], t1, t2, OP.subtract)
                nc.vector.tensor_tensor(rot[:, :, 32:64], t3, t4, OP.add)
                rots.append(rot)
            ps_v = psQ.tile([128, 512], f32, tag="pqkv", name="ps_v")
            for c in range(8):
                nc.tensor.matmul(
                    ps_v,
                    xt_full[:, si // 4, c,
                            (si % 4) * 128 : (si % 4) * 128 + 128],
                    wq_sb[:, 2, c, :],
                    start=(c == 0), stop=(c == 7),
                )
            nc.vector.tensor_copy(
                vaug[:, :, si, 0:Dh],
                ps_v.rearrange("p (h d) -> p h d", h=HL),
            )
            return rots

        def a_tail(si, rots):
            for dstT, rot in zip((qT, kT), rots):
                rotf = rot.rearrange("p h d -> p (h d)")
                ps_t = psQ.tile([128, 512], f32, tag="pqkv", name="ps_t")
                for jj in range(4):
                    nc.tensor.matmul(
                        ps_t[:, jj * 128 : (jj + 1) * 128],
                        rotf[:, jj * 128 : (jj + 1) * 128],
                        identb_sb, start=True, stop=True,
                    )
                if si < 4:
                    nc.scalar.copy(
                        dstT[:, :, si * 128 : (si + 1) * 128],
                        ps_t.rearrange("p (j s) -> p j s", j=4),
                    )
                else:
                    nc.vector.tensor_copy(
                        dstT[:, :, si * 128 : (si + 1) * 128],
                        ps_t.rearrange("p (j s) -> p j s", j=4),
                    )

        def b_pair(g, hh, zsb8):
            nj = 4 * (g + 1)
            px = [
                psX.tile([Dh + 1, 512], f32, tag="psx", name=f"px{hp}")
                for hp in range(2)
            ]
            for jj in range(0, nj, 2):
                ss = []
                for j in (jj, jj + 1):
                    diag = j >= 4 * g
                    c0 = 128 * (j - 4 * g) if diag else 0
                    ps_s = psS.tile([128, 2, 512], f32, tag="pss")
                    for hp in range(2):
                        nc.tensor.matmul(
                            ps_s[:, hp, c0:512],
                            kT[64 * hp : 64 * hp + 64, hh,
                               j * 128 : (j + 1) * 128],
                            qT[64 * hp : 64 * hp + 64, hh,
                               g * 512 + c0 : (g + 1) * 512],
                            start=True, stop=True,
                        )
                    pt = pt_pool.tile([128, 2, 512], bf16, tag="pt")
                    nc.scalar.activation(pt[:, :, c0:512], ps_s[:, :, c0:512],
                                         AT.Exp)
                    if diag:
                        for hp in range(2):
                            nc.vector.tensor_tensor(
                                pt[:, hp, c0 : c0 + 128],
                                pt[:, hp, c0 : c0 + 128],
                                keept_sb, OP.mult,
                            )
                    ss.append((j, c0, pt))
                for j, c0, pt in ss:
                    for hp in range(2):
                        nc.tensor.matmul(
                            px[hp][:, c0:512],
                            vaug[:, 2 * hh + hp, j, :], pt[:, hp, c0:512],
                            start=(j == 0), stop=(j == nj - 1),
                        )
            # stash Z rows into zsb8 (partition placement needs the gpsimd DMA)
            # and the unnormalized x into xus; normalization happens at the
            # group tail once all 8 heads' Z are in place.
            zta = nrm_pool.tile([1, 512], f32, tag="zta")
            nc.vector.tensor_copy(zta, px[0][Dh : Dh + 1, :])
            ztb = nrm_pool.tile([1, 512], f32, tag="ztb")
            nc.vector.tensor_copy(ztb, px[1][Dh : Dh + 1, :])
            nc.gpsimd.dma_start(zsb8[2 * hh : 2 * hh + 1, :], zta)
            nc.gpsimd.dma_start(zsb8[2 * hh + 1 : 2 * hh + 2, :], ztb)
            xus = xus_pool.tile([128, 512], bf16, tag="xus", name=f"xus{hh}")
            nc.vector.tensor_copy(xus[0:64, :], px[0][0:Dh, :])
            nc.vector.tensor_copy(xus[64:128, :], px[1][0:Dh, :])
            return xus

        def b_norm(g, zsb8, xus_l):
            # Z [8,512] -> columns [128,(m,h)] via K=8 transpose matmuls ->
            # reciprocal on fat partitions -> per-pair broadcast matmuls with
            # a stride-0 (free-dim broadcast) lhsT -> one DVE multiply per pair
            zc_t = psX.tile([128, 512], f32, tag="psx", name="zc_t")
            zcv = zc_t[:, 0:32].rearrange("p (m h) -> p m h", m=4)
            for m in range(4):
                nc.tensor.matmul(
                    zcv[:, m, :], zsb8[:, m * 128 : (m + 1) * 128],
                    identf_sb[0:HL, 0:HL], start=True, stop=True,
                )
            rcol8 = nrm_pool.tile([128, 4, HL], f32, tag="rcol")
            nc.vector.reciprocal(rcol8, zcv)
            for hh in range(4):
                bc_t = psX.tile([128, 512], f32, tag="psx", name="bc_t")
                for m in range(4):
                    for hp in range(2):
                        nc.tensor.matmul(
                            bc_t[64 * hp : 64 * hp + 64,
                                 m * 128 : (m + 1) * 128],
                            rcol8[:, m, 2 * hh + hp : 2 * hh + hp + 1]
                            .to_broadcast((128, 64)),
                            identf_sb, start=True, stop=True,
                        )
                nc.vector.tensor_tensor(
                    xT[:, hh, g * 512 : (g + 1) * 512], xus_l[hh], bc_t, OP.mult
                )

        def c_chunk(m):
            for half in range(2):
                ps_o = psQ.tile([128, 512], f32, tag="pqkv", name="ps_o")
                for p in range(4):
                    nc.tensor.matmul(
                        ps_o,
                        xT[:, p, m * 128 : (m + 1) * 128],
                        wout_sb[:, p, half * 512 : (half + 1) * 512],
                        start=(p == 0),
                        stop=(p == 3),
                    )
                ob = out_pool.tile([128, 512], f32, tag="ob")
                if m >= 12:
                    # epilogue: all exp work is done, ScalarE is idle
                    nc.scalar.copy(ob, ps_o)
                else:
                    nc.vector.tensor_copy(ob, ps_o)
                nc.sync.dma_start(
                    out_d[m * 128 : (m + 1) * 128,
                          half * 512 : (half + 1) * 512],
                    ob,
                )

        pend = None
        for si in range(4):
            r = a_proj(si)
            if pend is not None:
                a_tail(*pend)
            pend = (si, r)
        a_tail(*pend)
        for g in range(NG):
            zsb8 = nrm_pool.tile([HL, 512], f32, tag="zsb8")
            xus_l = []
            pend = None
            for hh in range(4):
                xus_l.append(b_pair(g, hh, zsb8))
                if pend is not None:
                    a_tail(*pend)
                    pend = None
                if g < NG - 1:
                    si = 4 * (g + 1) + hh
                    pend = (si, a_proj(si))
                if g >= 1:
                    c_chunk(4 * (g - 1) + hh)
            if pend is not None:
                a_tail(*pend)
            b_norm(g, zsb8, xus_l)
        for m in range(4 * (NG - 1), ST):
            c_chunk(m)

    nc.compile()
    return nc


def _numpy_fallback(x, w_q, w_k, w_v, w_out, seg, mask):
    """Exact numpy replica of the reference for non-causal masks."""
    frac = (2.0 * np.arange(Dh // 2, dtype=np.float32)) / Dh
    ts = (MAX_WAVELENGTH ** frac).astype(np.float32)

    def rope(t, pos):
        sinu = pos.astype(np.float32)[:, :, None] / ts  # [B,S,32]
        sn, cs = np.sin(sinu), np.cos(sinu)
        sn, cs = sn[:, :, None, :], cs[:, :, None, :]
        f, s_ = t[..., :32], t[..., 32:]
        return np.concatenate([f * cs - s_ * sn, s_ * cs + f * sn], -1)

    q = np.einsum("bsd,dhk->bshk", x, w_q)
    k = np.einsum("bsd,dhk->bshk", x, w_k)
    v = np.einsum("bsd,dhk->bshk", x, w_v)
    q, k = rope(q, seg), rope(k, seg)
    q = q / np.sqrt(np.float32(Dh))
    attn = np.einsum("bqhd,bkhd->bhqk", q, k)
    attn = np.where(mask, attn, np.finfo(np.float32).min)
    attn = attn - attn.max(-1, keepdims=True)
    e = np.exp(attn)
    attn = e / e.sum(-1, keepdims=True)
    xo = np.einsum("bhqk,bkhd->bqhd", attn, v)
    return np.einsum("bqhd,hdm->bqm", xo, w_out).astype(np.float32)


def _host_inputs(x, w_q, w_k, w_v, w_out, seg):
    frac = (2.0 * np.arange(Dh // 2, dtype=np.float32)) / Dh
    ts = (MAX_WAVELENGTH ** frac).astype(np.float32)
    identb = np.eye(128, dtype=np.float32).astype(BF16)
    keept = np.triu(np.ones((128, 128), dtype=np.float32)).astype(BF16)
    identf = np.eye(128, dtype=np.float32)

    in_maps = []
    for core in range(NCORES):
        b, g = core // 2, core % 2
        hs = slice(g * HL, (g + 1) * HL)
        wq_s = (w_q[:, hs, :] / np.float32(np.sqrt(Dh))).reshape(D, HL * Dh)
        wk_s = w_k[:, hs, :].reshape(D, HL * Dh)
        wv_s = w_v[:, hs, :].reshape(D, HL * Dh)
        wqkv = np.ascontiguousarray(
            np.concatenate([wq_s, wk_s, wv_s], axis=1), dtype=np.float32
        ).astype(BF16)
        woutp = np.stack(
            [
                w_out[g * HL + 2 * p : g * HL + 2 * p + 2].reshape(128, D)
                for p in range(4)
            ]
        ).astype(BF16)
        sinu = seg[b].astype(np.float32)[:, None] / ts  # [S, 32]
        # pre-arrange into partition-major SBUF layouts (contiguous DMAs)
        xt4 = (
            x[b].T.reshape(8, 128, 4, 512).transpose(1, 2, 0, 3)
        )  # [p, sc, c, s]
        wq4 = wqkv.astype(np.float32).reshape(8, 128, 3, 512).transpose(
            1, 2, 0, 3
        )  # [p, qkv, c, n]
        wo3 = woutp.transpose(1, 0, 2)  # [p, q, n]
        cos3 = np.cos(sinu).reshape(ST, 128, 32).transpose(1, 0, 2)
        sin3 = np.sin(sinu).reshape(ST, 128, 32).transpose(1, 0, 2)
        in_maps.append(
            {
                "xt": np.ascontiguousarray(xt4).astype(BF16),
                "wqkv": np.ascontiguousarray(wq4).astype(BF16),
                "woutp": np.ascontiguousarray(wo3).astype(BF16),
                "cost": np.ascontiguousarray(cos3).astype(BF16),
                "sint": np.ascontiguousarray(sin3).astype(BF16),
                "identb": identb,
                "keept": keept,
                "identf": identf,
            }
        )
    return in_maps


def _run(in_maps, trace=False):
    from concourse.bass_utils import run_bass_kernel_spmd

    if "nc" not in _CACHE:
        _CACHE["nc"] = _build_bass()
    return run_bass_kernel_spmd(
        _CACHE["nc"], in_maps, core_ids=list(range(NCORES)), trace=trace
    )


def kernel(**inputs):
    x = np.asarray(inputs["inputs"], dtype=np.float32)
    w_q = np.asarray(inputs["w_q"], dtype=np.float32)
    w_k = np.asarray(inputs["w_k"], dtype=np.float32)
    w_v = np.asarray(inputs["w_v"], dtype=np.float32)
    w_out = np.asarray(inputs["w_out"], dtype=np.float32)
    seg = np.asarray(inputs["segment_positions"])
    mask = np.asarray(inputs["mask"])

    causal = np.tril(np.ones((S, S), dtype=bool))
    if not all(np.array_equal(mask[b, 0], causal) for b in range(B)):
        return _numpy_fallback(x, w_q, w_k, w_v, w_out, seg, mask)

    in_maps = _host_inputs(x, w_q, w_k, w_v, w_out, seg)
    res = _run(in_maps)
    outs = [r_["out"] for r_ in res.results]
    result = np.empty((B, S, D), dtype=np.float32)
    for b in range(B):
        result[b] = outs[2 * b] + outs[2 * b + 1]
    return result


# revision 44
# speedup vs baseline: 1.1438x; 1.0068x over previous
"""Trainium2 Bass kernel for nn_AttentionBlock (B=4, S=2048, D=1024, H=16, Dh=64).

Sharding: 8 cores = 4 batches x 2 head-groups (8 heads each). Every core runs
the same Bass program on different input slices. The output projection is
row-sharded over head-groups, so the host sums the two partial outputs per
batch (the "all-reduce" of the sharding hint, done on host since we return
full outputs anyway).

Per-core pipeline (all matmuls bf16):
  A) QKV projection: lhsT = X^T chunks [128,128], rhs = Wqkv [128,1536 cols]
     -> psum [128(S-tile), 512] per q/k/v. The psum is copied to SBUF bf16
     (DVE), RoPE applied with bf16 2x-mode DVE ops (4 independent temps),
     then PE-transposed per head into qT/kT [Dh, S] packs. V goes to SBUF
     augmented with a ones column (V_aug [Sk,65]).
  B) Attention, two heads (one partition-half each) interleaved per k-tile:
     sT[Sk-tile 128, Sq 512] = kT_tile.T @ qT_group (alternating PE row
     groups so LDWEIGHTS hides). exp on ScalarE (PSUM->SBUF bf16). The
     causal mask is a [128,128] upper-tri multiply on DVE applied to the
     exp output of diagonal tiles only. AV: x_aug^T[65, Sq] += V_aug.T @ pT
     with row 64 accumulating the softmax denominator Z for free.
     Normalization: copy the two Z rows -> [2,512], reciprocal on DVE, one
     K=2 broadcast matmul -> [128,512] psum, single DVE multiply into xT.
  C) Output projection: out[Sq,512] += xT_pair.T @ WoutPair, DMA to HBM.

Issue order software-pipelines groups: QKV chunks of group g+1 and output
chunks of group g-1 are interleaved between attention head-pairs of group g
so the PE always has filler work while ScalarE runs exp.
"""

import sys

for _p in ("/opt/pypackages", "/opt/trn_rl_repo"):
    if _p not in sys.path:
        sys.path.insert(0, _p)

import numpy as np
import ml_dtypes

BF16 = ml_dtypes.bfloat16

B, S, D, H, Dh = 4, 2048, 1024, 16, 64
HL = H // 2          # heads per core
NCORES = 8
ST = S // 128        # 16 S-tiles of 128
NG = S // 512        # 4 q-groups of 512
MAX_WAVELENGTH = 10000.0

_CACHE = {}


def _build_bass():
    import concourse.bass as bass
    import concourse.mybir as mybir
    from concourse import bacc
    from concourse.tile import TileContext
    from contextlib import ExitStack

    f32 = mybir.dt.float32
    bf16 = mybir.dt.bfloat16
    AT = mybir.ActivationFunctionType
    OP = mybir.AluOpType

    nc = bacc.Bacc("TRN2", target_bir_lowering=False)

    # All inputs arrive pre-arranged on the host into the exact SBUF layouts
    # (partition-major, contiguous per partition) so every input DMA moves
    # large contiguous runs instead of 1KB gather descriptors.
    xt_d = nc.dram_tensor("xt", [128, 4, 8, 512], bf16, kind="ExternalInput")
    wqkv_d = nc.dram_tensor("wqkv", [128, 3, 8, 512], bf16, kind="ExternalInput")
    wout_d = nc.dram_tensor("woutp", [128, 4, D], bf16, kind="ExternalInput")
    cos_d = nc.dram_tensor("cost", [128, ST, Dh // 2], bf16, kind="ExternalInput")
    sin_d = nc.dram_tensor("sint", [128, ST, Dh // 2], bf16, kind="ExternalInput")
    identb_d = nc.dram_tensor("identb", [128, 128], bf16, kind="ExternalInput")
    keept_d = nc.dram_tensor("keept", [128, 128], bf16, kind="ExternalInput")
    identf_d = nc.dram_tensor("identf", [128, 128], f32, kind="ExternalInput")
    out_d = nc.dram_tensor("out", [S, D], f32, kind="ExternalOutput")

    with TileContext(nc) as tc, ExitStack() as ctx:
        consts = ctx.enter_context(tc.tile_pool(name="consts", bufs=1))
        persist = ctx.enter_context(tc.tile_pool(name="persist", bufs=1))

        # DMA order matters: what a_chunk(0) needs comes first so the PE can
        # start while the rest of the inputs stream in.
        identb_sb = consts.tile([128, 128], bf16, tag="identb")
        nc.sync.dma_start(identb_sb, identb_d[:, :])
        cos_sb = consts.tile([128, ST, 32], bf16, tag="cos")
        nc.sync.dma_start(cos_sb, cos_d[:, :, :])
        sin_sb = consts.tile([128, ST, 32], bf16, tag="sin")
        nc.sync.dma_start(sin_sb, sin_d[:, :, :])
        wq_sb = consts.tile([128, 3, 8, 512], bf16, tag="wqkv")
        xt_full = consts.tile([128, 4, 8, 512], bf16, tag="xtf")
        nc.sync.dma_start(wq_sb[:, 0, :, :], wqkv_d[:, 0, :, :])
        nc.sync.dma_start(xt_full[:, 0, :, :], xt_d[:, 0, :, :])
        nc.sync.dma_start(wq_sb[:, 1, :, :], wqkv_d[:, 1, :, :])
        nc.sync.dma_start(wq_sb[:, 2, :, :], wqkv_d[:, 2, :, :])
        for sc in range(1, 4):
            nc.sync.dma_start(xt_full[:, sc, :, :], xt_d[:, sc, :, :])
        keept_sb = consts.tile([128, 128], bf16, tag="keept")
        nc.sync.dma_start(keept_sb, keept_d[:, :])
        identf_sb = consts.tile([128, 128], f32, tag="identf")
        nc.sync.dma_start(identf_sb, identf_d[:, :])
        wout_sb = consts.tile([128, 4, 1024], bf16, tag="wout")
        nc.sync.dma_start(wout_sb, wout_d[:, :, :])

        qT = persist.tile([128, 4, S], bf16, tag="qT")
        kT = persist.tile([128, 4, S], bf16, tag="kT")
        xT = persist.tile([128, 4, S], bf16, tag="xT")
        vaug = persist.tile([128, HL, ST, Dh + 1], bf16, tag="vaug")
        nc.scalar.activation(
            vaug[:, :, :, Dh : Dh + 1],
            identb_sb[:, 0:1, None].to_broadcast((128, HL, ST, 1)),
            AT.Identity, bias=1.0, scale=0.0,
        )

        rw_pool = ctx.enter_context(tc.tile_pool(name="ropew", bufs=3))
        pt_pool = ctx.enter_context(tc.tile_pool(name="ptp", bufs=6))
        nrm_pool = ctx.enter_context(tc.tile_pool(name="nrm", bufs=2))
        xus_pool = ctx.enter_context(tc.tile_pool(name="xus", bufs=5))
        out_pool = ctx.enter_context(tc.tile_pool(name="outp", bufs=4))
        psQ = ctx.enter_context(tc.tile_pool(name="psQ", bufs=2, space="PSUM"))
        psS = ctx.enter_context(tc.tile_pool(name="psS", bufs=2, space="PSUM"))
        psX = ctx.enter_context(tc.tile_pool(name="psX", bufs=2, space="PSUM"))

        def a_proj(si):
            # q/k projection + RoPE (DVE) + v projection.  The PE-transposes
            # of the rotated q/k are issued later (a_tail) so other PE work
            # can fill the RoPE latency.
            cos_b = cos_sb[:, si, None, :].to_broadcast((128, HL, 32))
            sin_b = sin_sb[:, si, None, :].to_broadcast((128, HL, 32))
            rots = []
            for qkv in (0, 1):
                ps = psQ.tile([128, 512], f32, tag="pqkv", name=f"ps{qkv}")
                for c in range(8):
                    nc.tensor.matmul(
                        ps,
                        xt_full[:, si // 4, c,
                                (si % 4) * 128 : (si % 4) * 128 + 128],
                        wq_sb[:, qkv, c, :],
                        start=(c == 0), stop=(c == 7),
                    )
                xsb = rw_pool.tile([128, HL, Dh], bf16, tag="xsb")
                if si < 4:
                    # prologue: ScalarE is idle until the first exp
                    nc.scalar.copy(xsb, ps.rearrange("p (h d) -> p h d", h=HL))
                else:
                    nc.vector.tensor_copy(
                        xsb, ps.rearrange("p (h d) -> p h d", h=HL)
                    )
                x1, x2 = xsb[:, :, 0:32], xsb[:, :, 32:64]
                rot = rw_pool.tile([128, HL, Dh], bf16, tag="rot")
                t1 = rw_pool.tile([128, HL, 32], bf16, tag="t1")
                t2 = rw_pool.tile([128, HL, 32], bf16, tag="t2")
                t3 = rw_pool.tile([128, HL, 32], bf16, tag="t3")
                t4 = rw_pool.tile([128, HL, 32], bf16, tag="t4")
                nc.vector.tensor_tensor(t1, x1, cos_b, OP.mult)
                nc.vector.tensor_tensor(t2, x2, sin_b, OP.mult)
                nc.vector.tensor_tensor(t3, x1, sin_b, OP.mult)
                nc.vector.tensor_tensor(t4, x2, cos_b, OP.mult)
                nc.vector.tensor_tensor(rot[:, :, 0:32], t1, t2, OP.subtract)
                nc.vector.tensor_tensor(rot[:, :, 32:64], t3, t4, OP.add)
                rots.append(rot)
            ps_v = psQ.tile([128, 512], f32, tag="pqkv", name="ps_v")
            for c in range(8):
                nc.tensor.matmul(
                    ps_v,
                    xt_full[:, si // 4, c,
                            (si % 4) * 128 : (si % 4) * 128 + 128],
                    wq_sb[:, 2, c, :],
                    start=(c == 0), stop=(c == 7),
                )
            if si < 4:
                nc.scalar.copy(
                    vaug[:, :, si, 0:Dh],
                    ps_v.rearrange("p (h d) -> p h d", h=HL),
                )
            else:
                nc.vector.tensor_copy(
                    vaug[:, :, si, 0:Dh],
                    ps_v.rearrange("p (h d) -> p h d", h=HL),
                )
            return rots

        def a_tail(si, rots):
            for dstT, rot in zip((qT, kT), rots):
                rotf = rot.rearrange("p h d -> p (h d)")
                ps_t = psQ.tile([128, 512], f32, tag="pqkv", name="ps_t")
                for jj in range(4):
                    nc.tensor.matmul(
                        ps_t[:, jj * 128 : (jj + 1) * 128],
                        rotf[:, jj * 128 : (jj + 1) * 128],
                        identb_sb, start=True, stop=True,
                    )
                if si < 4:
                    nc.scalar.copy(
                        dstT[:, :, si * 128 : (si + 1) * 128],
                        ps_t.rearrange("p (j s) -> p j s", j=4),
                    )
                else:
                    nc.vector.tensor_copy(
                        dstT[:, :, si * 128 : (si + 1) * 128],
                        ps_t.rearrange("p (j s) -> p j s", j=4),
                    )

        def b_pair(g, hh, zsb8):
            nj = 4 * (g + 1)
            px = [
                psX.tile([Dh + 1, 512], f32, tag="psx", name=f"px{hp}")
                for hp in range(2)
            ]
            for jj in range(0, nj, 2):
                ss = []
                for j in (jj, jj + 1):
                    diag = j >= 4 * g
                    c0 = 128 * (j - 4 * g) if diag else 0
                    ps_s = psS.tile([128, 2, 512], f32, tag="pss")
                    for hp in range(2):
                        nc.tensor.matmul(
                            ps_s[:, hp, c0:512],
                            kT[64 * hp : 64 * hp + 64, hh,
                               j * 128 : (j + 1) * 128],
                            qT[64 * hp : 64 * hp + 64, hh,
                               g * 512 + c0 : (g + 1) * 512],
                            start=True, stop=True,
                        )
                    pt = pt_pool.tile([128, 2, 512], bf16, tag="pt")
                    nc.scalar.activation(pt[:, :, c0:512], ps_s[:, :, c0:512],
                                         AT.Exp)
                    if diag:
                        for hp in range(2):
                            nc.vector.tensor_tensor(
                                pt[:, hp, c0 : c0 + 128],
                                pt[:, hp, c0 : c0 + 128],
                                keept_sb, OP.mult,
                            )
                    ss.append((j, c0, pt))
                for j, c0, pt in ss:
                    for hp in range(2):
                        nc.tensor.matmul(
                            px[hp][:, c0:512],
                            vaug[:, 2 * hh + hp, j, :], pt[:, hp, c0:512],
                            start=(j == 0), stop=(j == nj - 1),
                        )
            # stash Z rows into zsb8 (partition placement needs the gpsimd DMA)
            # and the unnormalized x into xus; normalization happens at the
            # group tail once all 8 heads' Z are in place.
            zta = nrm_pool.tile([1, 512], f32, tag="zta")
            ztb = nrm_pool.tile([1, 512], f32, tag="ztb")
            if g == NG - 1 and hh == 3:
                # last pair of the last group: exp is done, ScalarE is idle,
                # and this chain is the exposed kernel tail
                nc.scalar.copy(zta, px[0][Dh : Dh + 1, :])
                nc.scalar.copy(ztb, px[1][Dh : Dh + 1, :])
            else:
                nc.vector.tensor_copy(zta, px[0][Dh : Dh + 1, :])
                nc.vector.tensor_copy(ztb, px[1][Dh : Dh + 1, :])
            nc.gpsimd.dma_start(zsb8[2 * hh : 2 * hh + 1, :], zta)
            nc.gpsimd.dma_start(zsb8[2 * hh + 1 : 2 * hh + 2, :], ztb)
            xus = xus_pool.tile([128, 512], bf16, tag="xus", name=f"xus{hh}")
            nc.vector.tensor_copy(xus[0:64, :], px[0][0:Dh, :])
            nc.vector.tensor_copy(xus[64:128, :], px[1][0:Dh, :])
            return xus

        def b_norm(g, zsb8, xus_l):
            # Z [8,512] -> columns [128,(m,h)] via K=8 transpose matmuls ->
            # reciprocal on fat partitions -> per-pair broadcast matmuls with
            # a stride-0 (free-dim broadcast) lhsT -> one DVE multiply per pair
            zc_t = psX.tile([128, 512], f32, tag="psx", name="zc_t")
            zcv = zc_t[:, 0:32].rearrange("p (m h) -> p m h", m=4)
            for m in range(4):
                nc.tensor.matmul(
                    zcv[:, m, :], zsb8[:, m * 128 : (m + 1) * 128],
                    identf_sb[0:HL, 0:HL], start=True, stop=True,
                )
            rcol8 = nrm_pool.tile([128, 4, HL], f32, tag="rcol")
            nc.vector.reciprocal(rcol8, zcv)
            for hh in range(4):
                bc_t = psX.tile([128, 512], f32, tag="psx", name="bc_t")
                for m in range(4):
                    for hp in range(2):
                        nc.tensor.matmul(
                            bc_t[64 * hp : 64 * hp + 64,
                                 m * 128 : (m + 1) * 128],
                            rcol8[:, m, 2 * hh + hp : 2 * hh + hp + 1]
                            .to_broadcast((128, 64)),
                            identf_sb, start=True, stop=True,
                        )
                nc.vector.tensor_tensor(
                    xT[:, hh, g * 512 : (g + 1) * 512], xus_l[hh], bc_t, OP.mult
                )

        def c_chunk(m):
            for half in range(2):
                ps_o = psQ.tile([128, 512], f32, tag="pqkv", name="ps_o")
                for p in range(4):
                    nc.tensor.matmul(
                        ps_o,
                        xT[:, p, m * 128 : (m + 1) * 128],
                        wout_sb[:, p, half * 512 : (half + 1) * 512],
                        start=(p == 0),
                        stop=(p == 3),
                    )
                ob = out_pool.tile([128, 512], f32, tag="ob")
                if m >= 12:
                    # epilogue: all exp work is done, ScalarE is idle
                    nc.scalar.copy(ob, ps_o)
                else:
                    nc.vector.tensor_copy(ob, ps_o)
                nc.sync.dma_start(
                    out_d[m * 128 : (m + 1) * 128,
                          half * 512 : (half + 1) * 512],
                    ob,
                )

        pend = None
        for si in range(4):
            r = a_proj(si)
            if pend is not None:
                a_tail(*pend)
            pend = (si, r)
        a_tail(*pend)
        for g in range(NG):
            zsb8 = nrm_pool.tile([HL, 512], f32, tag="zsb8")
            xus_l = []
            pend = None
            for hh in range(4):
                xus_l.append(b_pair(g, hh, zsb8))
                if pend is not None:
                    a_tail(*pend)
                    pend = None
                if g < NG - 1:
                    si = 4 * (g + 1) + hh
                    pend = (si, a_proj(si))
                if g >= 1:
                    c_chunk(4 * (g - 1) + hh)
            if pend is not None:
                a_tail(*pend)
            b_norm(g, zsb8, xus_l)
        for m in range(4 * (NG - 1), ST):
            c_chunk(m)

    nc.compile()
    return nc


def _numpy_fallback(x, w_q, w_k, w_v, w_out, seg, mask):
    """Exact numpy replica of the reference for non-causal masks."""
    frac = (2.0 * np.arange(Dh // 2, dtype=np.float32)) / Dh
    ts = (MAX_WAVELENGTH ** frac).astype(np.float32)

    def rope(t, pos):
        sinu = pos.astype(np.float32)[:, :, None] / ts  # [B,S,32]
        sn, cs = np.sin(sinu), np.cos(sinu)
        sn, cs = sn[:, :, None, :], cs[:, :, None, :]
        f, s_ = t[..., :32], t[..., 32:]
        return np.concatenate([f * cs - s_ * sn, s_ * cs + f * sn], -1)

    q = np.einsum("bsd,dhk->bshk", x, w_q)
    k = np.einsum("bsd,dhk->bshk", x, w_k)
    v = np.einsum("bsd,dhk->bshk", x, w_v)
    q, k = rope(q, seg), rope(k, seg)
    q = q / np.sqrt(np.float32(Dh))
    attn = np.einsum("bqhd,bkhd->bhqk", q, k)
    attn = np.where(mask, attn, np.finfo(np.float32).min)
    attn = attn - attn.max(-1, keepdims=True)
    e = np.exp(attn)
    attn = e / e.sum(-1, keepdims=True)
    xo = np.einsum("bhqk,bkhd->bqhd", attn, v)
    return np.einsum("bqhd,hdm->bqm", xo, w_out).astype(np.float32)


def _host_inputs(x, w_q, w_k, w_v, w_out, seg):
    frac = (2.0 * np.arange(Dh // 2, dtype=np.float32)) / Dh
    ts = (MAX_WAVELENGTH ** frac).astype(np.float32)
    identb = np.eye(128, dtype=np.float32).astype(BF16)
    keept = np.triu(np.ones((128, 128), dtype=np.float32)).astype(BF16)
    identf = np.eye(128, dtype=np.float32)

    in_maps = []
    for core in range(NCORES):
        b, g = core // 2, core % 2
        hs = slice(g * HL, (g + 1) * HL)
        wq_s = (w_q[:, hs, :] / np.float32(np.sqrt(Dh))).reshape(D, HL * Dh)
        wk_s = w_k[:, hs, :].reshape(D, HL * Dh)
        wv_s = w_v[:, hs, :].reshape(D, HL * Dh)
        wqkv = np.ascontiguousarray(
            np.concatenate([wq_s, wk_s, wv_s], axis=1), dtype=np.float32
        ).astype(BF16)
        woutp = np.stack(
            [
                w_out[g * HL + 2 * p : g * HL + 2 * p + 2].reshape(128, D)
                for p in range(4)
            ]
        ).astype(BF16)
        sinu = seg[b].astype(np.float32)[:, None] / ts  # [S, 32]
        # pre-arrange into partition-major SBUF layouts (contiguous DMAs)
        xt4 = (
            x[b].T.reshape(8, 128, 4, 512).transpose(1, 2, 0, 3)
        )  # [p, sc, c, s]
        wq4 = wqkv.astype(np.float32).reshape(8, 128, 3, 512).transpose(
            1, 2, 0, 3
        )  # [p, qkv, c, n]
        wo3 = woutp.transpose(1, 0, 2)  # [p, q, n]
        cos3 = np.cos(sinu).reshape(ST, 128, 32).transpose(1, 0, 2)
        sin3 = np.sin(sinu).reshape(ST, 128, 32).transpose(1, 0, 2)
        in_maps.append(
            {
                "xt": np.ascontiguousarray(xt4).astype(BF16),
                "wqkv": np.ascontiguousarray(wq4).astype(BF16),
                "woutp": np.ascontiguousarray(wo3).astype(BF16),
                "cost": np.ascontiguousarray(cos3).astype(BF16),
                "sint": np.ascontiguousarray(sin3).astype(BF16),
                "identb": identb,
                "keept": keept,
                "identf": identf,
            }
        )
    return in_maps


def _run(in_maps, trace=False):
    from concourse.bass_utils import run_bass_kernel_spmd

    if "nc" not in _CACHE:
        _CACHE["nc"] = _build_bass()
    return run_bass_kernel_spmd(
        _CACHE["nc"], in_maps, core_ids=list(range(NCORES)), trace=trace
    )


def kernel(**inputs):
    x = np.asarray(inputs["inputs"], dtype=np.float32)
    w_q = np.asarray(inputs["w_q"], dtype=np.float32)
    w_k = np.asarray(inputs["w_k"], dtype=np.float32)
    w_v = np.asarray(inputs["w_v"], dtype=np.float32)
    w_out = np.asarray(inputs["w_out"], dtype=np.float32)
    seg = np.asarray(inputs["segment_positions"])
    mask = np.asarray(inputs["mask"])

    causal = np.tril(np.ones((S, S), dtype=bool))
    if not all(np.array_equal(mask[b, 0], causal) for b in range(B)):
        return _numpy_fallback(x, w_q, w_k, w_v, w_out, seg, mask)

    in_maps = _host_inputs(x, w_q, w_k, w_v, w_out, seg)
    res = _run(in_maps)
    outs = [r_["out"] for r_ in res.results]
    result = np.empty((B, S, D), dtype=np.float32)
    for b in range(B):
        result[b] = outs[2 * b] + outs[2 * b + 1]
    return result
